# revision 25
# baseline (speedup 1.0000x reference)
"""NeRF-style render kernel for TRN2 (8 NeuronCores, data-parallel over rays).

Self-contained: hardcodes all shapes. Both MLPs run in float32r (1 cycle/row
on PE for moving dim >= 256). Posenc args are built as mid*B + C rank-1
per-ray matrices pre-scaled by 1/2pi; range reduction is a 2-op round
(magic-number) + subtract, with the 2pi fold done by the activation
engine's scale parameter. Fine trunk is scheduled layer-major across the
chunk so PE pipelines without relu stalls. Exp/sigmoid are batched to
minimize activation-table reloads.
"""
import os
import sys

sys.path.insert(0, '/opt/trn_rl_repo')
import numpy as np
import concourse.bass as bass
import concourse.bacc as bacc
import concourse.tile as tile
import concourse.mybir as mybir
from concourse.bass_utils import run_bass_kernel_spmd

F32 = mybir.dt.float32
F32R = mybir.dt.float32r
AF = mybir.ActivationFunctionType
OP = mybir.AluOpType

NCORES = 8
R = 128          # rays per core
S = 128          # samples per pass
CHUNK_RAYS = 16  # rays per chunk
NCHUNK = R // CHUNK_RAYS          # 8
CN = CHUNK_RAYS * S               # 2048 cols per chunk
TILE_N = 512                      # matmul moving size
NTILE = CN // TILE_N              # 4 point-tiles per chunk

MAGIC = np.float32(12582912.0)    # 1.5 * 2^23 (round-to-int trick)
TWOPI = float(np.float32(2.0 * np.pi))
INV2PI = 1.0 / (2.0 * np.pi)      # folded into posenc matrices (fp64 host)

BUILD_STAGE = int(os.environ.get("KERNEL_STAGE", "3"))
DEBUG_OUT = os.environ.get("KERNEL_DEBUG", "0") == "1"


# ---------------------------------------------------------------- host prep
def _posenc_rows(nf, span=None, minp=None):
    """A3 [6*nf,3] / const [6*nf] for rows f-major: per f: 3 sin, 3 cos."""
    rows = 6 * nf
    A3 = np.zeros((rows, 3), np.float64)
    ph = np.zeros((rows,), np.float64)
    for f in range(nf):
        for k in range(6):
            r = 6 * f + k
            d = k % 3
            sc = 2.0 ** f
            if span is not None:
                A3[r, d] = sc / span[d]
                ph[r] = -sc * minp[d] / span[d]
            else:
                A3[r, d] = sc
            if k >= 3:
                ph[r] += np.pi / 2.0
    return A3, ph


def host_prep(inp):
    c = {}
    f32 = np.float32

    # ---- coarse: arg rows = [60 sin-args (pre /2pi), 3 raw xyz] ----
    A3s, phs = _posenc_rows(10)
    cA3 = np.concatenate([A3s * INV2PI, np.eye(3)], 0)           # [63,3]
    cph = np.concatenate([phs * INV2PI, np.zeros(3)], 0)         # [63]
    c['cA3T'] = cA3.T.astype(f32).copy()                         # [3,63]
    c['cA4T'] = np.concatenate([cA3, cph[:, None]], 1).T.astype(f32).copy()

    # ---- fine: rows [60 sinx, 4 pad, 36 sinapp, 3 xyz, 3 applin] ----
    minp = inp['min_point'].astype(np.float64)
    span = (inp['max_point'] - inp['min_point']).astype(np.float64)
    A3a, pha = _posenc_rows(6, span=span, minp=minp)
    pad4 = np.zeros((4, 3))
    fA3 = np.concatenate([A3s * INV2PI, pad4, A3a * INV2PI,
                          np.eye(3), np.diag(1.0 / span)], 0)    # [106,3]
    fph = np.concatenate([phs * INV2PI, np.zeros(4), pha * INV2PI,
                          np.zeros(3), -minp / span], 0)
    c['fA3T'] = fA3.T.astype(f32).copy()                         # [3,106]
    c['fA4T'] = np.concatenate([fA3, fph[:, None]], 1).T.astype(f32).copy()

    # per-ray enc matrices (lhsT) for viewdir/time features
    Ad = np.zeros((24, 4), np.float64)
    for f in range(4):
        for k in range(6):
            r = 6 * f + k
            Ad[r, k % 3] = 2.0 ** f
            if k >= 3:
                Ad[r, 3] = np.pi / 2.0
    c['AdT'] = Ad.T.astype(f32).copy()                           # [4,24]
    At = np.zeros((12, 2), np.float64)
    for f in range(6):
        At[2 * f, 0] = 2.0 ** f
        At[2 * f + 1, 0] = 2.0 ** f
        At[2 * f + 1, 1] = np.pi / 2.0
    c['AtT'] = At.T.astype(f32).copy()                           # [2,12]

    # coarse MLP weights: single K=63 input layer [60 sin | 3 xyz]
    c['pW0ext'] = np.concatenate([inp['pW0'][3:63], inp['pW0'][0:3]], 0).copy()
    c['pW1'] = inp['pW1'].copy()
    c['pW2'] = inp['pW2'].copy()
    c['pWo'] = inp['pWo'].copy()                                 # [128,1]
    c['pb0col'] = inp['pb0'].reshape(-1, 1).copy()
    c['pb1col'] = inp['pb1'].reshape(-1, 1).copy()
    c['pb2col'] = inp['pb2'].reshape(-1, 1).copy()

    # fine MLP weights padded to K=106 feature layout
    def ext106(Wsin60, Wlin3, width):
        out = np.zeros((106, width), f32)
        out[0:60] = Wsin60
        out[100:103] = Wlin3
        return out

    c['fW0ext'] = ext106(inp['fW0'][3:63], inp['fW0'][0:3], 256)
    c['fWs_e_ext'] = ext106(inp['fWs'][256 + 3:256 + 63],
                            inp['fWs'][256:256 + 3], 256)

    def pack_km(Wm):  # [256, 256] -> [128, 4, 128], slot 2k+m
        out = np.zeros((128, 4, 128), f32)
        for k in range(2):
            for m in range(2):
                out[:, 2 * k + m, :] = Wm[k * 128:(k + 1) * 128,
                                          m * 128:(m + 1) * 128]
        return out

    for i in range(3):
        c[f'fWm{i}'] = pack_km(inp['fWm'][i])
        c[f'fWp{i}'] = pack_km(inp['fWp'][i])
    c['fWs_h'] = pack_km(inp['fWs'][0:256])
    c['fb0col'] = inp['fb0'].reshape(2, 128).T.copy()            # [128,2]
    for i in range(3):
        c[f'fbm{i}col'] = inp['fbm'][i].reshape(2, 128).T.copy()
        c[f'fbp{i}col'] = inp['fbp'][i].reshape(2, 128).T.copy()
    c['fbscol'] = inp['fbs'].reshape(2, 128).T.copy()

    # view head: fold Wfeat into Wview
    Wv = inp['Wview']
    Wv_d, Wv_emb, Wv_t, Wv_app = (Wv[256:283], Wv[283:331],
                                  Wv[331:344], Wv[344:383])
    Wfc = (inp['Wfeat'].astype(np.float64) @ Wv[0:256].astype(np.float64)
           ).astype(f32)
    out = np.zeros((128, 2, 128), f32)
    out[:, 0, :] = Wfc[0:128]
    out[:, 1, :] = Wfc[128:256]
    c['Wfc'] = out
    c['bveffcol'] = (inp['bfeat'].astype(np.float64)
                     @ Wv[0:256].astype(np.float64)
                     + inp['bview'].astype(np.float64)
                     ).astype(f32).reshape(-1, 1)
    # app-enc weights padded to K=106 rows [64:100 sin | 103:106 linear]
    Wva = np.zeros((106, 128), f32)
    Wva[64:100] = Wv_app[3:39]
    Wva[103:106] = Wv_app[0:3]
    c['fWv_app'] = Wva
    c['Wv_d_lin'] = np.ascontiguousarray(Wv_d[0:3])
    c['Wv_d_sin'] = np.ascontiguousarray(Wv_d[3:27])
    c['Wv_emb'] = np.ascontiguousarray(Wv_emb)
    c['Wv_t_lin'] = np.ascontiguousarray(Wv_t[0:1])
    c['Wv_t_sin'] = np.ascontiguousarray(Wv_t[1:13])
    c['Wsig'] = np.stack([inp['Wsig'][0:128, 0],
                          inp['Wsig'][128:256, 0]], 1).copy()    # [128,2]
    c['Wrgb'] = inp['Wrgb'].copy()                               # [128,3]
    c['emb_table'] = inp['emb_table'].copy()

    c['sgrid'] = np.broadcast_to(
        np.arange(129, dtype=f32) / 128.0, (128, 129)).copy()
    c['identity'] = np.eye(128, dtype=f32)
    E = np.zeros((4, 512), f32)
    for rl in range(4):
        E[rl, rl * 128:(rl + 1) * 128] = 1.0
    c['Etile'] = E
    c['iotacol'] = np.arange(100, dtype=f32).reshape(-1, 1)
    scalars = dict(pbo_f=float(inp['pbo'][0]), bsig_f=float(inp['bsig'][0]),
                   brgb=[float(x) for x in inp['brgb']])
    return c, scalars


INPUT_SHAPES = {
    'rays': (R, 12),
    'cA3T': (3, 63), 'cA4T': (4, 63),
    'fA3T': (3, 106), 'fA4T': (4, 106),
    'AdT': (4, 24), 'AtT': (2, 12),
    'pW0ext': (63, 128),
    'pW1': (128, 128), 'pW2': (128, 128), 'pWo': (128, 1),
    'pb0col': (128, 1), 'pb1col': (128, 1), 'pb2col': (128, 1),
    'fW0ext': (106, 256), 'fWm0': (128, 4, 128), 'fWm1': (128, 4, 128),
    'fWm2': (128, 4, 128), 'fWp0': (128, 4, 128), 'fWp1': (128, 4, 128),
    'fWp2': (128, 4, 128), 'fWs_h': (128, 4, 128), 'fWs_e_ext': (106, 256),
    'fb0col': (128, 2), 'fbm0col': (128, 2), 'fbm1col': (128, 2),
    'fbm2col': (128, 2), 'fbp0col': (128, 2), 'fbp1col': (128, 2),
    'fbp2col': (128, 2), 'fbscol': (128, 2),
    'Wfc': (128, 2, 128), 'bveffcol': (128, 1), 'fWv_app': (106, 128),
    'Wv_d_lin': (3, 128), 'Wv_d_sin': (24, 128), 'Wv_emb': (48, 128),
    'Wv_t_lin': (1, 128), 'Wv_t_sin': (12, 128),
    'Wsig': (128, 2), 'Wrgb': (128, 3),
    'emb_table': (100, 48),
    'sgrid': (128, 129), 'identity': (128, 128),
    'Etile': (4, 512), 'iotacol': (100, 1),
}
F32R_WEIGHTS = {'fW0ext', 'fWm0', 'fWm1', 'fWm2', 'fWp0', 'fWp1', 'fWp2',
                'fWs_h', 'fWs_e_ext', 'Wfc', 'fWv_app', 'Wv_d_lin',
                'Wv_d_sin', 'Wv_emb', 'Wv_t_lin', 'Wv_t_sin', 'Wsig', 'Wrgb',
                'emb_table', 'Etile'}


# ---------------------------------------------------------------- bass build
def build_nc(pbo_f, bsig_f, brgb, stage=3, debug=False):
    nc = bacc.Bacc("TRN2", target_bir_lowering=False)
    D = {k: nc.dram_tensor(k, list(v), F32, kind="ExternalInput")
         for k, v in INPUT_SHAPES.items()}
    OUT = nc.dram_tensor("rgb_out", [R, 3], F32, kind="ExternalOutput")
    dbg = {}
    if debug:
        for nm, shp in [("d_sigc", (R, S)), ("d_zf", (R, S + 1)),
                        ("d_wc", (R, S)), ("d_sigf", (R, S)),
                        ("d_wf", (R, S)), ("d_arg", (106, CN))]:
            dbg[nm] = nc.dram_tensor(nm, list(shp), F32, kind="ExternalOutput")
    with tile.TileContext(nc) as tc:
        _body(nc, tc, D, OUT, dbg, pbo_f, bsig_f, brgb, stage, debug)
    nc.compile()
    return nc


def _body(nc, tc, D, OUT, dbg, pbo_f, bsig_f, brgb, stage, debug):
    from contextlib import ExitStack
    ctx = ExitStack()
    wpool = ctx.enter_context(tc.tile_pool(name="w", bufs=1))
    per = ctx.enter_context(tc.tile_pool(name="per", bufs=1))
    pp2 = ctx.enter_context(tc.tile_pool(name="pp2", bufs=2))
    big = ctx.enter_context(tc.tile_pool(name="big", bufs=2))
    hp = ctx.enter_context(tc.tile_pool(name="h", bufs=2))
    dram = ctx.enter_context(tc.tile_pool(name="dr", bufs=2, space="DRAM"))
    psA = ctx.enter_context(tc.tile_pool(name="psA", bufs=4, space="PSUM"))
    psS = ctx.enter_context(tc.tile_pool(name="psS", bufs=1, space="PSUM"))
    psR = ctx.enter_context(tc.tile_pool(name="psR", bufs=1, space="PSUM"))
    psC = ctx.enter_context(tc.tile_pool(name="psC", bufs=1, space="PSUM"))

    W = {}
    EARLY = ['rays', 'identity', 'sgrid', 'cA3T', 'cA4T', 'AdT', 'AtT',
             'iotacol', 'emb_table', 'Wv_d_lin', 'Wv_d_sin', 'Wv_emb',
             'Wv_t_lin', 'Wv_t_sin', 'bveffcol', 'fA3T', 'fA4T',
             'pW0ext', 'pb0col', 'pW1', 'pb1col', 'pW2', 'pb2col', 'pWo']
    order = EARLY + [k for k in D if k not in EARLY]
    for k in order:
        t = D[k]
        if k == 'rays':
            continue
        dt = F32R if k in F32R_WEIGHTS else F32
        tl = wpool.tile(list(t.shape), dt, tag="w_" + k, name="w_" + k)
        nc.sync.dma_start(tl[:], t[:].bitcast(F32R) if dt == F32R else t[:])
        W[k] = tl
    rays = wpool.tile([R, 12], F32, tag="w_rays")
    nc.sync.dma_start(rays[:], D['rays'][:])
    ident = W['identity']

    # ---------------- phase 0: per-ray prep (ray-major layout)
    nearc = per.tile([R, 1], F32)
    nc.vector.tensor_scalar(nearc[:], rays[:, 6:7], 1e-8, None, op0=OP.max)
    spanc = per.tile([R, 1], F32)
    nc.vector.tensor_tensor(spanc[:], rays[:, 7:8], nearc[:], op=OP.subtract)

    dsq = per.tile([R, 3], F32)
    nc.vector.tensor_tensor(dsq[:], rays[:, 3:6], rays[:, 3:6], op=OP.mult)
    ssum = per.tile([R, 1], F32)
    nc.vector.reduce_sum(ssum[:], dsq[:], axis=mybir.AxisListType.X)
    norm = per.tile([R, 1], F32)
    nc.scalar.activation(norm[:], ssum[:], AF.Sqrt)
    for it in range(2):
        t1 = per.tile([R, 1], F32, tag="nwt")
        nc.vector.reciprocal(t1[:], norm[:])
        nc.vector.scalar_tensor_tensor(t1[:], ssum[:], 1.0, t1[:],
                                       op0=OP.mult, op1=OP.mult)
        nc.vector.tensor_tensor(t1[:], t1[:], norm[:], op=OP.add)
        nc.vector.tensor_scalar(norm[:], t1[:], 0.5, None, op0=OP.mult)
    invn = per.tile([R, 1], F32)
    nc.vector.reciprocal(invn[:], norm[:])

    # bundle: 0:3 o, 3 ones | 4:7 dir | 8:11 viewdir, 11 ones |
    #         12 t, 13 ones, 14 embid
    bundle = per.tile([R, 20], F32)
    nc.gpsimd.memset(bundle[:], 0.0)
    nc.vector.tensor_copy(bundle[:, 0:3], rays[:, 0:3])
    nc.vector.memset(bundle[:, 3:4], 1.0)
    nc.vector.tensor_copy(bundle[:, 4:7], rays[:, 3:6])
    nc.vector.tensor_scalar(bundle[:, 8:11], rays[:, 3:6], invn[:], None,
                            op0=OP.mult)
    nc.vector.memset(bundle[:, 11:12], 1.0)
    nc.vector.tensor_copy(bundle[:, 12:13], rays[:, 8:9])
    nc.vector.memset(bundle[:, 13:14], 1.0)
    nc.vector.tensor_copy(bundle[:, 14:15], rays[:, 9:10])

    def transp(col, nm):
        p = psC.tile([4, 128], F32, tag="ptp")
        nc.tensor.transpose(p[:], bundle[:, col:col + 4], ident[:])
        sb = per.tile([4, 128], F32, tag="tp_" + nm)
        nc.scalar.copy(sb[:], p[:])
        return sb

    oT = transp(0, "o")      # [oT;ones]
    dirT = transp(4, "d")
    vdT = transp(8, "vd")    # [viewdirT;ones]
    tT = transp(12, "t")     # [t;ones;embid]
    eiT = transp(14, "ei")   # row0 = embid

    def mm_copy(lhsT, rhs, shape, nm, dst_dtype=F32):
        p = psC.tile(shape, F32, tag="pmc")
        nc.tensor.matmul(p[:], lhsT, rhs, start=True, stop=True)
        sb = per.tile(shape, dst_dtype, tag="mc_" + nm)
        nc.scalar.copy(sb[:], p[:])
        return sb

    # per-ray rank-1 posenc matrices (pre-scaled by 1/2pi on sin rows)
    Bc = mm_copy(W['cA3T'][:], dirT[0:3, :], [63, 128], "Bc")
    Cc = mm_copy(W['cA4T'][:], oT[:], [63, 128], "Cc")
    Bf = mm_copy(W['fA3T'][:], dirT[0:3, :], [106, 128], "Bf")
    Cf = mm_copy(W['fA4T'][:], oT[:], [106, 128], "Cf")

    def rangered_v(ap, shape, tag):
        sc = per.tile(shape, F32, tag=tag)
        nc.vector.tensor_scalar(sc[:], ap, float(INV2PI), float(MAGIC),
                                op0=OP.mult, op1=OP.add)
        nc.vector.tensor_scalar(sc[:], sc[:], float(MAGIC), None,
                                op0=OP.subtract)
        nc.vector.scalar_tensor_tensor(ap, sc[:], -TWOPI, ap,
                                       op0=OP.mult, op1=OP.add)

    # per-ray view features
    argd = mm_copy(W['AdT'][:], vdT[:], [24, 128], 'argd')
    rangered_v(argd[:], [24, 128], "rrd")
    sind = per.tile([24, 128], F32R)
    nc.scalar.activation(sind[:], argd[:], AF.Sin)
    vd_r = per.tile([4, 128], F32R)
    nc.vector.tensor_copy(vd_r[:], vdT[:])

    argt = mm_copy(W['AtT'][:], tT[0:2, :], [12, 128], 'argt')
    rangered_v(argt[:], [12, 128], "rrt")
    sint = per.tile([12, 128], F32R)
    nc.scalar.activation(sint[:], argt[:], AF.Sin)
    t_r = per.tile([4, 128], F32R)
    nc.vector.tensor_copy(t_r[:], tT[:])

    embBC = per.tile([100, 128], F32)
    nc.gpsimd.partition_broadcast(embBC[:], eiT[0:1, :], channels=100)
    onehot = per.tile([100, 128], F32R)
    nc.vector.tensor_scalar(onehot[:], embBC[:], W['iotacol'][:], None,
                            op0=OP.is_equal)
    embT = mm_copy(W['emb_table'][:], onehot[:], [48, 128], 'embT',
                   dst_dtype=F32R)

    phv = psC.tile([128, 128], F32, tag="pmc")
    nc.tensor.matmul(phv[:], W['Wv_d_lin'][:], vd_r[0:3, :],
                     start=True, stop=False)
    nc.tensor.matmul(phv[:], W['Wv_d_sin'][:], sind[:], start=False, stop=False)
    nc.tensor.matmul(phv[:], W['Wv_emb'][:], embT[:], start=False, stop=False)
    nc.tensor.matmul(phv[:], W['Wv_t_lin'][:], t_r[0:1, :],
                     start=False, stop=False)
    nc.tensor.matmul(phv[:], W['Wv_t_sin'][:], sint[:], start=False, stop=True)
    hvray = per.tile([128, 128], F32)
    nc.vector.tensor_scalar(hvray[:], phv[:], W['bveffcol'][:], None,
                            op0=OP.add)
    phvT = psC.tile([128, 128], F32, tag="pmc")
    nc.tensor.transpose(phvT[:], hvray[:], ident[:])
    hvrayT = per.tile([128, 128], F32R)
    nc.scalar.copy(hvrayT[:], phvT[:])
    hvb = dram.tile([128, 128], F32R, tag="hvb")
    nc.sync.dma_start(hvb[:], hvrayT[:])
    hvre = wpool.tile([4, 32, 128], F32R, tag="hvre")
    nc.sync.dma_start(hvre[:], hvb[:].rearrange("(t rl) m -> rl t m", rl=4))

    # coarse z edges
    zc = per.tile([R, S + 1], F32)
    nc.vector.tensor_scalar(zc[:], W['sgrid'][:], spanc[:], None, op0=OP.mult)
    nc.vector.tensor_scalar(zc[:], zc[:], nearc[:], None, op0=OP.add)
    midc = per.tile([R, S], F32)
    nc.vector.tensor_tensor(midc[:], zc[:, 0:S], zc[:, 1:S + 1], op=OP.add)
    nc.vector.tensor_scalar(midc[:], midc[:], 0.5, None, op0=OP.mult)

    # posenc arg builder: arg = mid*B + C; RR+sin on rows 0:nsin in place.
    def build_pre(mid_src, r0, Bm, Cm, nrows, nsin, fdt):
        mbc = dram.tile([CHUNK_RAYS, S], F32, tag="midb")
        nc.sync.dma_start(mbc[:], mid_src[r0:r0 + CHUNK_RAYS, :])
        mfc = pp2.tile([1, CN], F32, tag="flat", bufs=1)
        nc.sync.dma_start(mfc[:],
                          mbc[:].rearrange("p f -> (p f)").unsqueeze(0))
        mBC = big.tile([nrows, CN], F32, tag="mbc", bufs=1)
        nc.gpsimd.partition_broadcast(mBC[:], mfc[:], channels=nrows)
        arg = big.tile([nrows, CN], F32, tag="argt", bufs=1)
        b3 = Bm[:, r0:r0 + CHUNK_RAYS].unsqueeze(2).broadcast_to(
            [nrows, CHUNK_RAYS, S])
        c3 = Cm[:, r0:r0 + CHUNK_RAYS].unsqueeze(2).broadcast_to(
            [nrows, CHUNK_RAYS, S])
        a3 = arg[:].rearrange("p (r s) -> p r s", r=CHUNK_RAYS)
        m3 = mBC[:].rearrange("p (r s) -> p r s", r=CHUNK_RAYS)
        nc.vector.tensor_tensor(a3, m3, b3, op=OP.mult)           # DVE
        nc.gpsimd.tensor_tensor(a3, a3, c3, op=OP.add)            # Pool
        # range reduction on sin rows: k = round(a); a -= k; sin(2pi*a)
        kk = big.tile([nsin, CN], F32, tag="kk", bufs=1)
        nc.vector.tensor_scalar(kk[:], arg[0:nsin, :], float(MAGIC),
                                float(MAGIC), op0=OP.add,
                                op1=OP.subtract)                  # DVE
        if fdt == F32:   # coarse: r on DVE (Pool is posenc-bound)
            nc.vector.tensor_tensor(arg[0:nsin, :], arg[0:nsin, :], kk[:],
                                    op=OP.subtract)
        else:            # fine: r on Pool (DVE is trunk-bound)
            nc.gpsimd.tensor_tensor(arg[0:nsin, :], arg[0:nsin, :], kk[:],
                                    op=OP.subtract)
        return arg

    def build_sin(arg, nrows, nsin, fdt):
        ef = big.tile([nrows, CN], fdt, tag="eft")
        nc.scalar.activation(ef[0:nsin, :], arg[0:nsin, :], AF.Sin,
                             scale=TWOPI)                         # Act
        if nrows > nsin:
            src_ap = arg[nsin:nrows, :]
            nc.sync.dma_start(ef[nsin:nrows, :],
                              src_ap.bitcast(F32R) if fdt == F32R else src_ap)
        return ef

    # ======================= COARSE PASS =======================
    sigcT = per.tile([R, S], F32, tag="sigcT")
    arg_nx = build_pre(midc, 0, Bc, Cc, 63, 60, F32)
    ef_nx = build_sin(arg_nx, 63, 60, F32)
    for ci in range(NCHUNK):
        r0 = ci * CHUNK_RAYS
        rhs = ef_nx
        sb_ = dram.tile([1, CN], F32, tag="sigb")
        sigflat = pp2.tile([1, CN], F32, tag="sigflat", bufs=1)
        # layer-major across the 4 tiles so PE never stalls on relus
        ch1 = hp.tile([128, CN], F32, tag="ch1")
        for t in range(NTILE):
            cols = slice(t * TILE_N, (t + 1) * TILE_N)
            p1 = psA.tile([128, TILE_N], F32, tag="mmps")
            nc.tensor.matmul(p1[:], W['pW0ext'][:], rhs[:, cols],
                             start=True, stop=True)
            if t < 3:
                nc.scalar.activation(ch1[:, cols], p1[:], AF.Relu,
                                     bias=W['pb0col'][:])
            else:
                nc.vector.tensor_scalar(ch1[:, cols], p1[:], W['pb0col'][:],
                                        0.0, op0=OP.add, op1=OP.max)
        if ci + 1 < NCHUNK:
            arg_nx = build_pre(midc, r0 + CHUNK_RAYS, Bc, Cc, 63, 60, F32)
        ch2 = hp.tile([128, CN], F32, tag="ch2", bufs=1)
        for t in range(NTILE):
            cols = slice(t * TILE_N, (t + 1) * TILE_N)
            p2 = psA.tile([128, TILE_N], F32, tag="mmps")
            nc.tensor.matmul(p2[:], W['pW1'][:], ch1[:, cols],
                             start=True, stop=True)
            if t < 3:
                nc.scalar.activation(ch2[:, cols], p2[:], AF.Relu,
                                     bias=W['pb1col'][:])
            else:
                nc.vector.tensor_scalar(ch2[:, cols], p2[:], W['pb1col'][:],
                                        0.0, op0=OP.add, op1=OP.max)
        if ci + 1 < NCHUNK:
            ef_nx = build_sin(arg_nx, 63, 60, F32)
        ch3 = hp.tile([128, CN], F32, tag="ch1")
        for t in range(NTILE):
            cols = slice(t * TILE_N, (t + 1) * TILE_N)
            p3 = psA.tile([128, TILE_N], F32, tag="mmps")
            nc.tensor.matmul(p3[:], W['pW2'][:], ch2[:, cols],
                             start=True, stop=True)
            if t < 3:
                nc.scalar.activation(ch3[:, cols], p3[:], AF.Relu,
                                     bias=W['pb2col'][:])
            else:
                nc.vector.tensor_scalar(ch3[:, cols], p3[:], W['pb2col'][:],
                                        0.0, op0=OP.add, op1=OP.max)
        for t in range(NTILE):
            cols = slice(t * TILE_N, (t + 1) * TILE_N)
            ps_ = psS.tile([1, TILE_N], F32, tag="sigps")
            nc.tensor.matmul(ps_[:], W['pWo'][:], ch3[:, cols],
                             start=True, stop=True)
            if t % 2 == 0:
                nc.scalar.copy(sigflat[0:1, cols], ps_[:])
            else:
                nc.vector.tensor_copy(sigflat[0:1, cols], ps_[:])
        nc.sync.dma_start(sb_[:], sigflat[:])
        nc.sync.dma_start(sigcT[r0:r0 + CHUNK_RAYS, :],
                          sb_[:].rearrange("a (p f) -> (a p) f", p=CHUNK_RAYS))

    if debug:
        nc.sync.dma_start(dbg["d_sigc"][:], sigcT[:])
    if stage < 2:
        ctx.close()
        return

    # ======================= raw2weights helper =======================
    def raw2w(sigT_ap, z_lo, z_hi, norm_ap, bias_f, nrows, tag):
        """w = alpha * exclusive-cumprod(1-alpha+1e-10); returns (w, dz)."""
        P = nrows
        dz = per.tile([P, S], F32, tag=tag + "dz")
        nc.vector.tensor_tensor(dz[:], z_hi, z_lo, op=OP.subtract)
        di = per.tile([P, S], F32, tag=tag + "di")
        nc.vector.tensor_scalar(di[:], dz[:], norm_ap, None, op0=OP.mult)
        s1 = per.tile([P, S], F32, tag=tag + "s1")
        nc.vector.tensor_scalar(s1[:], sigT_ap, bias_f, 0.0,
                                op0=OP.add, op1=OP.max)
        ea = per.tile([P, S], F32, tag=tag + "ea")
        nc.vector.tensor_tensor(ea[:], s1[:], di[:], op=OP.mult)
        e = per.tile([P, S], F32, tag=tag + "e")
        nc.scalar.activation(e[:], ea[:], AF.Exp, scale=-1.0)
        al = per.tile([P, S], F32, tag=tag + "al")
        nc.vector.tensor_scalar(al[:], e[:], -1.0, 1.0, op0=OP.mult, op1=OP.add)
        om = per.tile([P, S], F32, tag=tag + "om")
        nc.vector.tensor_scalar(om[:], e[:], 1e-10, None, op0=OP.add)
        tr = per.tile([P, S], F32, tag=tag + "tr")
        nc.vector.tensor_tensor_scan(tr[:], om[:], om[:], 1.0,
                                     op0=OP.mult, op1=OP.bypass)
        w = per.tile([P, S], F32, tag=tag + "w")
        nc.vector.tensor_copy(w[:, 0:1], al[:, 0:1])
        nc.vector.tensor_tensor(w[:, 1:S], al[:, 1:S], tr[:, 0:S - 1],
                                op=OP.mult)
        return w, dz

    # ================= inverse-CDF sampling (all 128 rays) =================
    wc, dzc = raw2w(sigcT[:, :], zc[:, 0:S], zc[:, 1:S + 1],
                    norm[:, :], pbo_f, R, "c")
    if debug:
        nc.sync.dma_start(dbg["d_wc"][:], wc[:])
    Wt = per.tile([R, S], F32)
    nc.vector.tensor_scalar(Wt[:], wc[:], 1e-5, None, op0=OP.add)
    Sx = per.tile([R, S], F32)
    nc.vector.memset(Sx[:, 0:1], 0.0)
    nc.vector.tensor_tensor_scan(Sx[:, 1:S], Wt[:, 0:S - 1],
                                 Wt[:, 0:S - 1], 0.0,
                                 op0=OP.add, op1=OP.bypass)
    Tt = per.tile([R, 1], F32)
    nc.vector.tensor_tensor(Tt[:], Sx[:, S - 1:S], Wt[:, S - 1:S], op=OP.add)
    P2 = per.tile([R, S], F32)
    nc.vector.reciprocal(P2[:], Wt[:])
    nc.vector.tensor_tensor(P2[:], P2[:], dzc[:], op=OP.mult)
    Sn = Sx
    nc.vector.tensor_scalar(Sn[:], Sx[:], -1.0, None, op0=OP.mult)
    UT = per.tile([R, S + 1], F32)
    nc.vector.tensor_scalar(UT[:], W['sgrid'][:], Tt[:], None, op0=OP.mult)
    B2 = per.tile([R, S], F32)
    nc.vector.tensor_tensor(B2[:], Sn[:], P2[:], op=OP.mult)
    zf = per.tile([R, S + 1], F32)
    # split iterations: DVE-only path vs Pool-prep path (Pool builds x_)
    N_DVE = 50
    for j in range(S + 1):
        if j < N_DVE:
            x_ = pp2.tile([R, S], F32, tag="pdfx", name="x_")
            nc.vector.scalar_tensor_tensor(x_[:], Sn[:], UT[:, j:j + 1],
                                           P2[:], op0=OP.add, op1=OP.mult)
        else:
            x1 = pp2.tile([R, S], F32, tag="pdfx1", name="x1", bufs=3)
            nc.gpsimd.tensor_scalar(x1[:], P2[:], UT[:, j:j + 1], None,
                                    op0=OP.mult)
            x_ = pp2.tile([R, S], F32, tag="pdfx2", name="x_", bufs=3)
            nc.gpsimd.tensor_tensor(x_[:], x1[:], B2[:], op=OP.add)
        sc_ = pp2.tile([R, S], F32, tag="pdfsc", name="sc_", bufs=1)
        nc.vector.scalar_tensor_tensor(sc_[:], x_[:], 0.0, dzc[:],
                                       op0=OP.max, op1=OP.min,
                                       accum_out=zf[:, j:j + 1])
    nc.vector.tensor_scalar(zf[:], zf[:], zc[:, 0:1], None, op0=OP.add)
    if debug:
        nc.sync.dma_start(dbg["d_zf"][:], zf[:])
    if stage < 3:
        ctx.close()
        return

    midf = per.tile([R, S], F32)
    nc.vector.tensor_tensor(midf[:], zf[:, 0:S], zf[:, 1:S + 1], op=OP.add)
    nc.vector.tensor_scalar(midf[:], midf[:], 0.5, None, op0=OP.mult)

    # ======================= FINE PASS =======================
    sigfT = per.tile([R, S], F32, tag="sigfT")
    Rt = per.tile([R, S], F32, tag="Rt")
    Gt = per.tile([R, S], F32, tag="Gt")
    Bt = per.tile([R, S], F32, tag="Bt")
    rgb_rows = [Rt, Gt, Bt]

    arg_nx = build_pre(midf, 0, Bf, Cf, 106, 100, F32R)
    ef_nx = build_sin(arg_nx, 106, 100, F32R)
    for ci in range(NCHUNK):
        r0 = ci * CHUNK_RAYS
        ef = ef_nx
        if debug and ci == 0:
            nc.sync.dma_start(dbg["d_arg"][:], ef[:].bitcast(F32))

        # ---- trunk, layer-major across the chunk's 4 tiles ----
        def relu_to(psum, hout_ap, bias_ap, idx):
            if idx % 2 == 0:
                nc.scalar.activation(hout_ap, psum, AF.Relu, bias=bias_ap)
            else:
                nc.vector.tensor_scalar(hout_ap, psum, bias_ap, 0.0,
                                        op0=OP.add, op1=OP.max)

        h = hp.tile([128, 2 * CN], F32R, tag="fh")
        for t in range(NTILE):
            cols = slice(t * TILE_N, (t + 1) * TILE_N)
            for m in range(2):
                ps = psA.tile([128, TILE_N], F32, tag="mmps")
                nc.tensor.matmul(ps[:], W['fW0ext'][:, m * 128:(m + 1) * 128],
                                 ef[:, cols], start=True, stop=True)
                relu_to(ps[:], h[:, m * CN + t * TILE_N:
                                 m * CN + (t + 1) * TILE_N],
                        W['fb0col'][:, m:m + 1], m)

        if ci + 1 < NCHUNK:
            arg_nx = build_pre(midf, r0 + CHUNK_RAYS, Bf, Cf, 106, 100, F32R)

        def lm_layer(wname, bname, hin, skip=False):
            hout = hp.tile([128, 2 * CN], F32R, tag="fh")
            for t in range(NTILE):
                cs = slice(t * TILE_N, (t + 1) * TILE_N)
                for m in range(2):
                    ps = psA.tile([128, TILE_N], F32, tag="mmps")
                    nc.tensor.matmul(ps[:], W[wname][:, m, :],
                                     hin[:, t * TILE_N:(t + 1) * TILE_N],
                                     start=True, stop=False)
                    nc.tensor.matmul(ps[:], W[wname][:, 2 + m, :],
                                     hin[:, CN + t * TILE_N:
                                          CN + (t + 1) * TILE_N],
                                     start=False, stop=not skip)
                    if skip:
                        nc.tensor.matmul(
                            ps[:], W['fWs_e_ext'][:, m * 128:(m + 1) * 128],
                            ef[:, cs], start=False, stop=True)
                    relu_to(ps[:], hout[:, m * CN + t * TILE_N:
                                        m * CN + (t + 1) * TILE_N],
                            W[bname][:, m:m + 1], m)
            return hout

        h = lm_layer('fWm0', 'fbm0col', h)
        h = lm_layer('fWm1', 'fbm1col', h)
        if ci + 1 < NCHUNK:
            ef_nx = build_sin(arg_nx, 106, 100, F32R)
        h = lm_layer('fWm2', 'fbm2col', h)
        h = lm_layer('fWs_h', 'fbscol', h, skip=True)
        h = lm_layer('fWp0', 'fbp0col', h)
        h = lm_layer('fWp1', 'fbp1col', h)
        h = lm_layer('fWp2', 'fbp2col', h)

        # ---- heads: sigma + view + rgb, interleaved per tile ----
        rgbS = big.tile([3, CN], F32, tag="rgbS", bufs=1)
        sb_ = dram.tile([1, CN], F32, tag="sigb")
        sigflat = pp2.tile([1, CN], F32, tag="sigflat", bufs=1)
        for t in range(NTILE):
            cols = slice(t * TILE_N, (t + 1) * TILE_N)
            gtile = ci * NTILE + t
            ps_ = psS.tile([1, TILE_N], F32, tag="sigps")
            nc.tensor.matmul(ps_[:], W['Wsig'][:, 0:1],
                             h[:, t * TILE_N:(t + 1) * TILE_N],
                             start=True, stop=False)
            nc.tensor.matmul(ps_[:], W['Wsig'][:, 1:2],
                             h[:, CN + t * TILE_N:CN + (t + 1) * TILE_N],
                             start=False, stop=True)
            nc.scalar.copy(sigflat[0:1, cols], ps_[:])

            pv = psA.tile([128, TILE_N], F32, tag="mmps")
            nc.tensor.matmul(pv[:], W['Wfc'][:, 0, :],
                             h[:, t * TILE_N:(t + 1) * TILE_N],
                             start=True, stop=False)
            nc.tensor.matmul(pv[:], W['Wfc'][:, 1, :],
                             h[:, CN + t * TILE_N:CN + (t + 1) * TILE_N],
                             start=False, stop=False)
            nc.tensor.matmul(pv[:], W['fWv_app'][:], ef[:, cols],
                             start=False, stop=False)
            nc.tensor.matmul(pv[:], hvre[:, gtile, :], W['Etile'][:],
                             start=False, stop=True)
            hv = hp.tile([128, TILE_N], F32R, tag="fhv", bufs=2)
            if t % 2 == 0:
                nc.scalar.activation(hv[:], pv[:], AF.Relu)
            else:
                nc.vector.tensor_scalar(hv[:], pv[:], 0.0, None, op0=OP.max)

            prgb = psR.tile([3, TILE_N], F32, tag="rgbps")
            nc.tensor.matmul(prgb[:], W['Wrgb'][:], hv[:],
                             start=True, stop=True)
            nc.scalar.copy(rgbS[0:3, cols], prgb[:])
        nc.sync.dma_start(sb_[:], sigflat[:])
        nc.sync.dma_start(sigfT[r0:r0 + CHUNK_RAYS, :],
                          sb_[:].rearrange("a (p f) -> (a p) f", p=CHUNK_RAYS))
        rb_ = dram.tile([3, CN], F32, tag="rgbb")
        nc.sync.dma_start(rb_[:], rgbS[:])
        for cch in range(3):
            nc.sync.dma_start(
                rgb_rows[cch][r0:r0 + CHUNK_RAYS, :],
                rb_[cch:cch + 1, :].rearrange("a (p f) -> (a p) f",
                                              p=CHUNK_RAYS))

    # ======================= tail: composite =======================
    wf, _dzf = raw2w(sigfT[:, :], zf[:, 0:S], zf[:, 1:S + 1],
                     norm[:, :], bsig_f, R, "f")
    if debug:
        nc.sync.dma_start(dbg["d_sigf"][:], sigfT[:])
        nc.sync.dma_start(dbg["d_wf"][:], wf[:])
    rgbout = per.tile([R, 3], F32)
    for cch in range(3):
        sg = per.tile([R, S], F32, tag="sg%d" % cch)
        nc.scalar.activation(sg[:], rgb_rows[cch][:], AF.Sigmoid,
                             bias=float(brgb[cch]))
        nc.vector.tensor_tensor(sg[:], sg[:], wf[:], op=OP.mult)
        nc.vector.tensor_reduce(rgbout[:, cch:cch + 1], sg[:],
                                axis=mybir.AxisListType.X, op=OP.add)
    nc.sync.dma_start(OUT[:], rgbout[:])
    ctx.close()


# ---------------------------------------------------------------- entry
_CACHE = {}


def kernel(**inputs):
    inp = {k: np.asarray(v) for k, v in inputs.items()}
    consts, scal = host_prep(inp)
    key = (BUILD_STAGE, DEBUG_OUT, scal['pbo_f'], scal['bsig_f'],
           tuple(scal['brgb']))
    if key not in _CACHE:
        _CACHE[key] = build_nc(scal['pbo_f'], scal['bsig_f'], scal['brgb'],
                               stage=BUILD_STAGE, debug=DEBUG_OUT)
    nc = _CACHE[key]
    rays = np.asarray(inp['rays'], np.float32)
    in_maps = []
    for core in range(NCORES):
        m = {k: np.ascontiguousarray(v, dtype=np.float32)
             for k, v in consts.items()}
        m['rays'] = np.ascontiguousarray(rays[core * R:(core + 1) * R])
        in_maps.append(m)
    res = run_bass_kernel_spmd(nc, in_maps, core_ids=list(range(NCORES)))
    globals()['_LAST_RESULTS'] = res
    return np.concatenate([r['rgb_out'] for r in res.results], 0)


# revision 34
# speedup vs baseline: 1.0382x; 1.0382x over previous
"""NeRF-style render kernel for TRN2 (8 NeuronCores, data-parallel over rays).

Self-contained: hardcodes all shapes. Both MLPs run in float32r (1 cycle/row
on PE for moving dim >= 256). Posenc args are built as mid*B + C rank-1
per-ray matrices pre-scaled by 1/2pi; range reduction is a 2-op round
(magic-number) + subtract, with the 2pi fold done by the activation
engine's scale parameter. Fine trunk is scheduled layer-major across the
chunk so PE pipelines without relu stalls. Exp/sigmoid are batched to
minimize activation-table reloads.
"""
import os
import sys

sys.path.insert(0, '/opt/trn_rl_repo')
import numpy as np
import concourse.bass as bass
import concourse.bacc as bacc
import concourse.tile as tile
import concourse.mybir as mybir
from concourse.bass_utils import run_bass_kernel_spmd

F32 = mybir.dt.float32
F32R = mybir.dt.float32r
AF = mybir.ActivationFunctionType
OP = mybir.AluOpType

NCORES = 8
R = 128          # rays per core
S = 128          # samples per pass
CHUNK_RAYS = 16  # rays per chunk
NCHUNK = R // CHUNK_RAYS          # 8
CN = CHUNK_RAYS * S               # 2048 cols per chunk
TILE_N = 512                      # matmul moving size
NTILE = CN // TILE_N              # 4 point-tiles per chunk

MAGIC = np.float32(12582912.0)    # 1.5 * 2^23 (round-to-int trick)
TWOPI = float(np.float32(2.0 * np.pi))
INV2PI = 1.0 / (2.0 * np.pi)      # folded into posenc matrices (fp64 host)

BUILD_STAGE = int(os.environ.get("KERNEL_STAGE", "3"))
DEBUG_OUT = os.environ.get("KERNEL_DEBUG", "0") == "1"


# ---------------------------------------------------------------- host prep
def _posenc_rows(nf, span=None, minp=None):
    """A3 [6*nf,3] / const [6*nf] for rows f-major: per f: 3 sin, 3 cos."""
    rows = 6 * nf
    A3 = np.zeros((rows, 3), np.float64)
    ph = np.zeros((rows,), np.float64)
    for f in range(nf):
        for k in range(6):
            r = 6 * f + k
            d = k % 3
            sc = 2.0 ** f
            if span is not None:
                A3[r, d] = sc / span[d]
                ph[r] = -sc * minp[d] / span[d]
            else:
                A3[r, d] = sc
            if k >= 3:
                ph[r] += np.pi / 2.0
    return A3, ph


def host_prep(inp):
    c = {}
    f32 = np.float32

    # ---- coarse: arg rows = [60 sin-args (pre /2pi), 3 raw xyz] ----
    A3s, phs = _posenc_rows(10)
    cA3 = np.concatenate([A3s * INV2PI, np.eye(3)], 0)           # [63,3]
    cph = np.concatenate([phs * INV2PI, np.zeros(3)], 0)         # [63]
    c['cA3T'] = cA3.T.astype(f32).copy()                         # [3,63]
    c['cA4T'] = np.concatenate([cA3, cph[:, None]], 1).T.astype(f32).copy()

    # ---- fine: rows [60 sinx, 4 pad, 36 sinapp, 3 xyz, 3 applin] ----
    minp = inp['min_point'].astype(np.float64)
    span = (inp['max_point'] - inp['min_point']).astype(np.float64)
    A3a, pha = _posenc_rows(6, span=span, minp=minp)
    pad4 = np.zeros((4, 3))
    fA3 = np.concatenate([A3s * INV2PI, pad4, A3a * INV2PI,
                          np.eye(3), np.diag(1.0 / span)], 0)    # [106,3]
    fph = np.concatenate([phs * INV2PI, np.zeros(4), pha * INV2PI,
                          np.zeros(3), -minp / span], 0)
    c['fA3T'] = fA3.T.astype(f32).copy()                         # [3,106]
    c['fA4T'] = np.concatenate([fA3, fph[:, None]], 1).T.astype(f32).copy()

    # per-ray enc matrices (lhsT) for viewdir/time features
    Ad = np.zeros((24, 4), np.float64)
    for f in range(4):
        for k in range(6):
            r = 6 * f + k
            Ad[r, k % 3] = 2.0 ** f
            if k >= 3:
                Ad[r, 3] = np.pi / 2.0
    c['AdT'] = Ad.T.astype(f32).copy()                           # [4,24]
    At = np.zeros((12, 2), np.float64)
    for f in range(6):
        At[2 * f, 0] = 2.0 ** f
        At[2 * f + 1, 0] = 2.0 ** f
        At[2 * f + 1, 1] = np.pi / 2.0
    c['AtT'] = At.T.astype(f32).copy()                           # [2,12]

    # coarse MLP weights: single K=63 input layer [60 sin | 3 xyz]
    c['pW0ext'] = np.concatenate([inp['pW0'][3:63], inp['pW0'][0:3]], 0).copy()
    c['pW1'] = inp['pW1'].copy()
    c['pW2'] = inp['pW2'].copy()
    c['pWo'] = inp['pWo'].copy()                                 # [128,1]
    c['pb0col'] = inp['pb0'].reshape(-1, 1).copy()
    c['pb1col'] = inp['pb1'].reshape(-1, 1).copy()
    c['pb2col'] = inp['pb2'].reshape(-1, 1).copy()

    # fine MLP weights padded to K=106 feature layout
    def ext106(Wsin60, Wlin3, width):
        out = np.zeros((106, width), f32)
        out[0:60] = Wsin60
        out[100:103] = Wlin3
        return out

    c['fW0ext'] = ext106(inp['fW0'][3:63], inp['fW0'][0:3], 256)
    c['fWs_e_ext'] = ext106(inp['fWs'][256 + 3:256 + 63],
                            inp['fWs'][256:256 + 3], 256)

    def pack_km(Wm):  # [256, 256] -> [128, 4, 128], slot 2k+m
        out = np.zeros((128, 4, 128), f32)
        for k in range(2):
            for m in range(2):
                out[:, 2 * k + m, :] = Wm[k * 128:(k + 1) * 128,
                                          m * 128:(m + 1) * 128]
        return out

    for i in range(3):
        c[f'fWm{i}'] = pack_km(inp['fWm'][i])
        c[f'fWp{i}'] = pack_km(inp['fWp'][i])
    c['fWs_h'] = pack_km(inp['fWs'][0:256])
    c['fb0col'] = inp['fb0'].reshape(2, 128).T.copy()            # [128,2]
    for i in range(3):
        c[f'fbm{i}col'] = inp['fbm'][i].reshape(2, 128).T.copy()
        c[f'fbp{i}col'] = inp['fbp'][i].reshape(2, 128).T.copy()
    c['fbscol'] = inp['fbs'].reshape(2, 128).T.copy()

    # view head: fold Wfeat into Wview
    Wv = inp['Wview']
    Wv_d, Wv_emb, Wv_t, Wv_app = (Wv[256:283], Wv[283:331],
                                  Wv[331:344], Wv[344:383])
    Wfc = (inp['Wfeat'].astype(np.float64) @ Wv[0:256].astype(np.float64)
           ).astype(f32)
    out = np.zeros((128, 2, 128), f32)
    out[:, 0, :] = Wfc[0:128]
    out[:, 1, :] = Wfc[128:256]
    c['Wfc'] = out
    c['bveffcol'] = (inp['bfeat'].astype(np.float64)
                     @ Wv[0:256].astype(np.float64)
                     + inp['bview'].astype(np.float64)
                     ).astype(f32).reshape(-1, 1)
    # app-enc weights padded to K=106 rows [64:100 sin | 103:106 linear]
    Wva = np.zeros((106, 128), f32)
    Wva[64:100] = Wv_app[3:39]
    Wva[103:106] = Wv_app[0:3]
    c['fWv_app'] = Wva
    c['Wv_d_lin'] = np.ascontiguousarray(Wv_d[0:3])
    c['Wv_d_sin'] = np.ascontiguousarray(Wv_d[3:27])
    c['Wv_emb'] = np.ascontiguousarray(Wv_emb)
    c['Wv_t_lin'] = np.ascontiguousarray(Wv_t[0:1])
    c['Wv_t_sin'] = np.ascontiguousarray(Wv_t[1:13])
    c['Wsig'] = np.stack([inp['Wsig'][0:128, 0],
                          inp['Wsig'][128:256, 0]], 1).copy()    # [128,2]
    c['Wrgb'] = inp['Wrgb'].copy()                               # [128,3]
    c['emb_table'] = inp['emb_table'].copy()

    c['sgrid'] = np.broadcast_to(
        np.arange(129, dtype=f32) / 128.0, (128, 129)).copy()
    c['identity'] = np.eye(128, dtype=f32)
    E = np.zeros((4, 512), f32)
    for rl in range(4):
        E[rl, rl * 128:(rl + 1) * 128] = 1.0
    c['Etile'] = E
    c['iotacol'] = np.arange(100, dtype=f32).reshape(-1, 1)
    scalars = dict(pbo_f=float(inp['pbo'][0]), bsig_f=float(inp['bsig'][0]),
                   brgb=[float(x) for x in inp['brgb']])
    return c, scalars


INPUT_SHAPES = {
    'rays': (R, 12),
    'cA3T': (3, 63), 'cA4T': (4, 63),
    'fA3T': (3, 106), 'fA4T': (4, 106),
    'AdT': (4, 24), 'AtT': (2, 12),
    'pW0ext': (63, 128),
    'pW1': (128, 128), 'pW2': (128, 128), 'pWo': (128, 1),
    'pb0col': (128, 1), 'pb1col': (128, 1), 'pb2col': (128, 1),
    'fW0ext': (106, 256), 'fWm0': (128, 4, 128), 'fWm1': (128, 4, 128),
    'fWm2': (128, 4, 128), 'fWp0': (128, 4, 128), 'fWp1': (128, 4, 128),
    'fWp2': (128, 4, 128), 'fWs_h': (128, 4, 128), 'fWs_e_ext': (106, 256),
    'fb0col': (128, 2), 'fbm0col': (128, 2), 'fbm1col': (128, 2),
    'fbm2col': (128, 2), 'fbp0col': (128, 2), 'fbp1col': (128, 2),
    'fbp2col': (128, 2), 'fbscol': (128, 2),
    'Wfc': (128, 2, 128), 'bveffcol': (128, 1), 'fWv_app': (106, 128),
    'Wv_d_lin': (3, 128), 'Wv_d_sin': (24, 128), 'Wv_emb': (48, 128),
    'Wv_t_lin': (1, 128), 'Wv_t_sin': (12, 128),
    'Wsig': (128, 2), 'Wrgb': (128, 3),
    'emb_table': (100, 48),
    'sgrid': (128, 129), 'identity': (128, 128),
    'Etile': (4, 512), 'iotacol': (100, 1),
}
F32R_WEIGHTS = {'fW0ext', 'fWm0', 'fWm1', 'fWm2', 'fWp0', 'fWp1', 'fWp2',
                'fWs_h', 'fWs_e_ext', 'Wfc', 'fWv_app', 'Wv_d_lin',
                'Wv_d_sin', 'Wv_emb', 'Wv_t_lin', 'Wv_t_sin', 'Wsig', 'Wrgb',
                'emb_table', 'Etile'}


# ---------------------------------------------------------------- bass build
def build_nc(pbo_f, bsig_f, brgb, stage=3, debug=False):
    nc = bacc.Bacc("TRN2", target_bir_lowering=False)
    D = {k: nc.dram_tensor(k, list(v), F32, kind="ExternalInput")
         for k, v in INPUT_SHAPES.items()}
    OUT = nc.dram_tensor("rgb_out", [R, 3], F32, kind="ExternalOutput")
    dbg = {}
    if debug:
        for nm, shp in [("d_sigc", (R, S)), ("d_zf", (R, S + 1)),
                        ("d_wc", (R, S)), ("d_sigf", (R, S)),
                        ("d_wf", (R, S)), ("d_arg", (106, CN))]:
            dbg[nm] = nc.dram_tensor(nm, list(shp), F32, kind="ExternalOutput")
    with tile.TileContext(nc) as tc:
        _body(nc, tc, D, OUT, dbg, pbo_f, bsig_f, brgb, stage, debug)
    nc.compile()
    return nc


def _body(nc, tc, D, OUT, dbg, pbo_f, bsig_f, brgb, stage, debug):
    from contextlib import ExitStack
    ctx = ExitStack()
    wpool = ctx.enter_context(tc.tile_pool(name="w", bufs=1))
    per = ctx.enter_context(tc.tile_pool(name="per", bufs=1))
    pp2 = ctx.enter_context(tc.tile_pool(name="pp2", bufs=2))
    big = ctx.enter_context(tc.tile_pool(name="big", bufs=2))
    hp = ctx.enter_context(tc.tile_pool(name="h", bufs=2))
    dram = ctx.enter_context(tc.tile_pool(name="dr", bufs=2, space="DRAM"))
    psA = ctx.enter_context(tc.tile_pool(name="psA", bufs=4, space="PSUM"))
    psS = ctx.enter_context(tc.tile_pool(name="psS", bufs=1, space="PSUM"))
    psR = ctx.enter_context(tc.tile_pool(name="psR", bufs=1, space="PSUM"))
    psC = ctx.enter_context(tc.tile_pool(name="psC", bufs=1, space="PSUM"))

    W = {}
    EARLY = ['rays', 'identity', 'sgrid', 'cA3T', 'cA4T', 'AdT', 'AtT',
             'iotacol', 'emb_table', 'Wv_d_lin', 'Wv_d_sin', 'Wv_emb',
             'Wv_t_lin', 'Wv_t_sin', 'bveffcol', 'fA3T', 'fA4T',
             'pW0ext', 'pb0col', 'pW1', 'pb1col', 'pW2', 'pb2col', 'pWo']
    order = EARLY + [k for k in D if k not in EARLY]
    for k in order:
        t = D[k]
        if k == 'rays':
            continue
        dt = F32R if k in F32R_WEIGHTS else F32
        tl = wpool.tile(list(t.shape), dt, tag="w_" + k, name="w_" + k)
        nc.sync.dma_start(tl[:], t[:].bitcast(F32R) if dt == F32R else t[:])
        W[k] = tl
    rays = wpool.tile([R, 12], F32, tag="w_rays")
    nc.sync.dma_start(rays[:], D['rays'][:])
    ident = W['identity']

    # ---------------- phase 0: per-ray prep (ray-major layout)
    nearc = per.tile([R, 1], F32)
    nc.vector.tensor_scalar(nearc[:], rays[:, 6:7], 1e-8, None, op0=OP.max)
    spanc = per.tile([R, 1], F32)
    nc.vector.tensor_tensor(spanc[:], rays[:, 7:8], nearc[:], op=OP.subtract)

    dsq = per.tile([R, 3], F32)
    nc.vector.tensor_tensor(dsq[:], rays[:, 3:6], rays[:, 3:6], op=OP.mult)
    ssum = per.tile([R, 1], F32)
    nc.vector.reduce_sum(ssum[:], dsq[:], axis=mybir.AxisListType.X)
    norm = per.tile([R, 1], F32)
    nc.scalar.activation(norm[:], ssum[:], AF.Sqrt)
    for it in range(2):
        t1 = per.tile([R, 1], F32, tag="nwt")
        nc.vector.reciprocal(t1[:], norm[:])
        nc.vector.scalar_tensor_tensor(t1[:], ssum[:], 1.0, t1[:],
                                       op0=OP.mult, op1=OP.mult)
        nc.vector.tensor_tensor(t1[:], t1[:], norm[:], op=OP.add)
        nc.vector.tensor_scalar(norm[:], t1[:], 0.5, None, op0=OP.mult)
    invn = per.tile([R, 1], F32)
    nc.vector.reciprocal(invn[:], norm[:])

    # bundle: 0:3 o, 3 ones | 4:7 dir | 8:11 viewdir, 11 ones |
    #         12 t, 13 ones, 14 embid
    bundle = per.tile([R, 20], F32)
    nc.gpsimd.memset(bundle[:], 0.0)
    nc.vector.tensor_copy(bundle[:, 0:3], rays[:, 0:3])
    nc.vector.memset(bundle[:, 3:4], 1.0)
    nc.vector.tensor_copy(bundle[:, 4:7], rays[:, 3:6])
    nc.vector.tensor_scalar(bundle[:, 8:11], rays[:, 3:6], invn[:], None,
                            op0=OP.mult)
    nc.vector.memset(bundle[:, 11:12], 1.0)
    nc.vector.tensor_copy(bundle[:, 12:13], rays[:, 8:9])
    nc.vector.memset(bundle[:, 13:14], 1.0)
    nc.vector.tensor_copy(bundle[:, 14:15], rays[:, 9:10])

    def transp(col, nm):
        p = psC.tile([4, 128], F32, tag="ptp")
        nc.tensor.transpose(p[:], bundle[:, col:col + 4], ident[:])
        sb = per.tile([4, 128], F32, tag="tp_" + nm)
        nc.scalar.copy(sb[:], p[:])
        return sb

    oT = transp(0, "o")      # [oT;ones]
    dirT = transp(4, "d")
    vdT = transp(8, "vd")    # [viewdirT;ones]
    tT = transp(12, "t")     # [t;ones;embid]
    eiT = transp(14, "ei")   # row0 = embid

    def mm_copy(lhsT, rhs, shape, nm, dst_dtype=F32):
        p = psC.tile(shape, F32, tag="pmc")
        nc.tensor.matmul(p[:], lhsT, rhs, start=True, stop=True)
        sb = per.tile(shape, dst_dtype, tag="mc_" + nm)
        nc.scalar.copy(sb[:], p[:])
        return sb

    # per-ray rank-1 posenc matrices (pre-scaled by 1/2pi on sin rows)
    Bc = mm_copy(W['cA3T'][:], dirT[0:3, :], [63, 128], "Bc")
    Cc = mm_copy(W['cA4T'][:], oT[:], [63, 128], "Cc")
    Bf = mm_copy(W['fA3T'][:], dirT[0:3, :], [106, 128], "Bf")
    Cf = mm_copy(W['fA4T'][:], oT[:], [106, 128], "Cf")

    def rangered_v(ap, shape, tag):
        sc = per.tile(shape, F32, tag=tag)
        nc.vector.tensor_scalar(sc[:], ap, float(INV2PI), float(MAGIC),
                                op0=OP.mult, op1=OP.add)
        nc.vector.tensor_scalar(sc[:], sc[:], float(MAGIC), None,
                                op0=OP.subtract)
        nc.vector.scalar_tensor_tensor(ap, sc[:], -TWOPI, ap,
                                       op0=OP.mult, op1=OP.add)

    # per-ray view features
    argd = mm_copy(W['AdT'][:], vdT[:], [24, 128], 'argd')
    rangered_v(argd[:], [24, 128], "rrd")
    sind = per.tile([24, 128], F32R)
    nc.scalar.activation(sind[:], argd[:], AF.Sin)
    vd_r = per.tile([4, 128], F32R)
    nc.vector.tensor_copy(vd_r[:], vdT[:])

    argt = mm_copy(W['AtT'][:], tT[0:2, :], [12, 128], 'argt')
    rangered_v(argt[:], [12, 128], "rrt")
    sint = per.tile([12, 128], F32R)
    nc.scalar.activation(sint[:], argt[:], AF.Sin)
    t_r = per.tile([4, 128], F32R)
    nc.vector.tensor_copy(t_r[:], tT[:])

    embBC = per.tile([100, 128], F32)
    nc.gpsimd.partition_broadcast(embBC[:], eiT[0:1, :], channels=100)
    onehot = per.tile([100, 128], F32R)
    nc.vector.tensor_scalar(onehot[:], embBC[:], W['iotacol'][:], None,
                            op0=OP.is_equal)
    embT = mm_copy(W['emb_table'][:], onehot[:], [48, 128], 'embT',
                   dst_dtype=F32R)

    phv = psC.tile([128, 128], F32, tag="pmc")
    nc.tensor.matmul(phv[:], W['Wv_d_lin'][:], vd_r[0:3, :],
                     start=True, stop=False)
    nc.tensor.matmul(phv[:], W['Wv_d_sin'][:], sind[:], start=False, stop=False)
    nc.tensor.matmul(phv[:], W['Wv_emb'][:], embT[:], start=False, stop=False)
    nc.tensor.matmul(phv[:], W['Wv_t_lin'][:], t_r[0:1, :],
                     start=False, stop=False)
    nc.tensor.matmul(phv[:], W['Wv_t_sin'][:], sint[:], start=False, stop=True)
    hvray = per.tile([128, 128], F32)
    nc.vector.tensor_scalar(hvray[:], phv[:], W['bveffcol'][:], None,
                            op0=OP.add)
    phvT = psC.tile([128, 128], F32, tag="pmc")
    nc.tensor.transpose(phvT[:], hvray[:], ident[:])
    hvrayT = per.tile([128, 128], F32R)
    nc.scalar.copy(hvrayT[:], phvT[:])
    hvb = dram.tile([128, 128], F32R, tag="hvb")
    nc.sync.dma_start(hvb[:], hvrayT[:])
    hvre = wpool.tile([4, 32, 128], F32R, tag="hvre")
    nc.sync.dma_start(hvre[:], hvb[:].rearrange("(t rl) m -> rl t m", rl=4))

    # coarse z edges
    zc = per.tile([R, S + 1], F32)
    nc.vector.tensor_scalar(zc[:], W['sgrid'][:], spanc[:], None, op0=OP.mult)
    nc.vector.tensor_scalar(zc[:], zc[:], nearc[:], None, op0=OP.add)
    midc = per.tile([R, S], F32)
    nc.vector.tensor_tensor(midc[:], zc[:, 0:S], zc[:, 1:S + 1], op=OP.add)
    nc.vector.tensor_scalar(midc[:], midc[:], 0.5, None, op0=OP.mult)

    # posenc arg builder: arg = mid*B + C; RR+sin on rows 0:nsin in place.
    def build_args(mid_src, r0, Bm, Cm, nrows, nsin, fdt):
        mbc = dram.tile([CHUNK_RAYS, S], F32, tag="midb")
        nc.sync.dma_start(mbc[:], mid_src[r0:r0 + CHUNK_RAYS, :])
        mfc = pp2.tile([1, CN], F32, tag="flat", bufs=1)
        nc.sync.dma_start(mfc[:],
                          mbc[:].rearrange("p f -> (p f)").unsqueeze(0))
        mBC = big.tile([nrows, CN], F32, tag="mbc", bufs=1)
        nc.gpsimd.partition_broadcast(mBC[:], mfc[:], channels=nrows)
        arg = big.tile([nrows, CN], F32, tag="argt", bufs=1)
        b3 = Bm[:, r0:r0 + CHUNK_RAYS].unsqueeze(2).broadcast_to(
            [nrows, CHUNK_RAYS, S])
        c3 = Cm[:, r0:r0 + CHUNK_RAYS].unsqueeze(2).broadcast_to(
            [nrows, CHUNK_RAYS, S])
        a3 = arg[:].rearrange("p (r s) -> p r s", r=CHUNK_RAYS)
        m3 = mBC[:].rearrange("p (r s) -> p r s", r=CHUNK_RAYS)
        nc.vector.tensor_tensor(a3, m3, b3, op=OP.mult)           # DVE
        nc.gpsimd.tensor_tensor(a3, a3, c3, op=OP.add)            # Pool
        # range reduction on sin rows: k = round(a); a -= k; sin(2pi*a)
        kk = big.tile([nsin, CN], F32, tag="kk", bufs=1)
        nc.vector.tensor_scalar(kk[:], arg[0:nsin, :], float(MAGIC),
                                float(MAGIC), op0=OP.add,
                                op1=OP.subtract)                  # DVE
        if fdt == F32:   # coarse: r on DVE (Pool is posenc-bound)
            nc.vector.tensor_tensor(arg[0:nsin, :], arg[0:nsin, :], kk[:],
                                    op=OP.subtract)
        else:            # fine: r on Pool (DVE is trunk-bound)
            nc.gpsimd.tensor_tensor(arg[0:nsin, :], arg[0:nsin, :], kk[:],
                                    op=OP.subtract)
        ef = big.tile([nrows, CN], fdt, tag="eft")
        nc.scalar.activation(ef[0:nsin, :], arg[0:nsin, :], AF.Sin,
                             scale=TWOPI)                         # Act
        if nrows > nsin:
            src_ap = arg[nsin:nrows, :]
            nc.sync.dma_start(ef[nsin:nrows, :],
                              src_ap.bitcast(F32R) if fdt == F32R else src_ap)
        return ef

    # ======================= COARSE PASS =======================
    sigcT = per.tile([R, S], F32, tag="sigcT")
    for ci in range(NCHUNK):
        r0 = ci * CHUNK_RAYS
        rhs = build_args(midc, r0, Bc, Cc, 63, 60, F32)
        sb_ = dram.tile([1, CN], F32, tag="sigb")
        sigflat = pp2.tile([1, CN], F32, tag="sigflat", bufs=1)
        # layer-major across the 4 tiles so PE never stalls on relus
        ch1 = hp.tile([128, CN], F32, tag="ch1")
        for t in range(NTILE):
            cols = slice(t * TILE_N, (t + 1) * TILE_N)
            p1 = psA.tile([128, TILE_N], F32, tag="mmps")
            nc.tensor.matmul(p1[:], W['pW0ext'][:], rhs[:, cols],
                             start=True, stop=True)
            if t < 3:
                nc.scalar.activation(ch1[:, cols], p1[:], AF.Relu,
                                     bias=W['pb0col'][:])
            else:
                nc.vector.tensor_scalar(ch1[:, cols], p1[:], W['pb0col'][:],
                                        0.0, op0=OP.add, op1=OP.max)
        ch2 = hp.tile([128, CN], F32, tag="ch2", bufs=1)
        for t in range(NTILE):
            cols = slice(t * TILE_N, (t + 1) * TILE_N)
            p2 = psA.tile([128, TILE_N], F32, tag="mmps")
            nc.tensor.matmul(p2[:], W['pW1'][:], ch1[:, cols],
                             start=True, stop=True)
            if t < 3:
                nc.scalar.activation(ch2[:, cols], p2[:], AF.Relu,
                                     bias=W['pb1col'][:])
            else:
                nc.vector.tensor_scalar(ch2[:, cols], p2[:], W['pb1col'][:],
                                        0.0, op0=OP.add, op1=OP.max)
        ch3 = hp.tile([128, CN], F32, tag="ch1")
        for t in range(NTILE):
            cols = slice(t * TILE_N, (t + 1) * TILE_N)
            p3 = psA.tile([128, TILE_N], F32, tag="mmps")
            nc.tensor.matmul(p3[:], W['pW2'][:], ch2[:, cols],
                             start=True, stop=True)
            if t < 3:
                nc.scalar.activation(ch3[:, cols], p3[:], AF.Relu,
                                     bias=W['pb2col'][:])
            else:
                nc.vector.tensor_scalar(ch3[:, cols], p3[:], W['pb2col'][:],
                                        0.0, op0=OP.add, op1=OP.max)
        for t in range(NTILE):
            cols = slice(t * TILE_N, (t + 1) * TILE_N)
            ps_ = psS.tile([1, TILE_N], F32, tag="sigps")
            nc.tensor.matmul(ps_[:], W['pWo'][:], ch3[:, cols],
                             start=True, stop=True)
            if t % 2 == 0:
                nc.scalar.copy(sigflat[0:1, cols], ps_[:])
            else:
                nc.vector.tensor_copy(sigflat[0:1, cols], ps_[:])
        nc.sync.dma_start(sb_[:], sigflat[:])
        nc.sync.dma_start(sigcT[r0:r0 + CHUNK_RAYS, :],
                          sb_[:].rearrange("a (p f) -> (a p) f", p=CHUNK_RAYS))

    if debug:
        nc.sync.dma_start(dbg["d_sigc"][:], sigcT[:])
    if stage < 2:
        ctx.close()
        return

    # ======================= raw2weights helper =======================
    def raw2w(sigT_ap, z_lo, z_hi, norm_ap, bias_f, nrows, tag):
        """w = alpha * exclusive-cumprod(1-alpha+1e-10); returns (w, dz)."""
        P = nrows
        dz = per.tile([P, S], F32, tag=tag + "dz")
        nc.vector.tensor_tensor(dz[:], z_hi, z_lo, op=OP.subtract)
        di = per.tile([P, S], F32, tag=tag + "di")
        nc.vector.tensor_scalar(di[:], dz[:], norm_ap, None, op0=OP.mult)
        s1 = per.tile([P, S], F32, tag=tag + "s1")
        nc.vector.tensor_scalar(s1[:], sigT_ap, bias_f, 0.0,
                                op0=OP.add, op1=OP.max)
        ea = per.tile([P, S], F32, tag=tag + "ea")
        nc.vector.tensor_tensor(ea[:], s1[:], di[:], op=OP.mult)
        e = per.tile([P, S], F32, tag=tag + "e")
        nc.scalar.activation(e[:], ea[:], AF.Exp, scale=-1.0)
        al = per.tile([P, S], F32, tag=tag + "al")
        nc.vector.tensor_scalar(al[:], e[:], -1.0, 1.0, op0=OP.mult, op1=OP.add)
        om = per.tile([P, S], F32, tag=tag + "om")
        nc.vector.tensor_scalar(om[:], e[:], 1e-10, None, op0=OP.add)
        tr = per.tile([P, S], F32, tag=tag + "tr")
        nc.vector.tensor_tensor_scan(tr[:], om[:], om[:], 1.0,
                                     op0=OP.mult, op1=OP.bypass)
        w = per.tile([P, S], F32, tag=tag + "w")
        nc.vector.tensor_copy(w[:, 0:1], al[:, 0:1])
        nc.vector.tensor_tensor(w[:, 1:S], al[:, 1:S], tr[:, 0:S - 1],
                                op=OP.mult)
        return w, dz

    # ================= inverse-CDF sampling (all 128 rays) =================
    wc, dzc = raw2w(sigcT[:, :], zc[:, 0:S], zc[:, 1:S + 1],
                    norm[:, :], pbo_f, R, "c")
    if debug:
        nc.sync.dma_start(dbg["d_wc"][:], wc[:])
    Wt = per.tile([R, S], F32)
    nc.vector.tensor_scalar(Wt[:], wc[:], 1e-5, None, op0=OP.add)
    Sx = per.tile([R, S], F32)
    nc.vector.memset(Sx[:, 0:1], 0.0)
    nc.vector.tensor_tensor_scan(Sx[:, 1:S], Wt[:, 0:S - 1],
                                 Wt[:, 0:S - 1], 0.0,
                                 op0=OP.add, op1=OP.bypass)
    Tt = per.tile([R, 1], F32)
    nc.vector.tensor_tensor(Tt[:], Sx[:, S - 1:S], Wt[:, S - 1:S], op=OP.add)
    P2 = per.tile([R, S], F32)
    nc.vector.reciprocal(P2[:], Wt[:])
    nc.vector.tensor_tensor(P2[:], P2[:], dzc[:], op=OP.mult)
    Sn = Sx
    nc.vector.tensor_scalar(Sn[:], Sx[:], -1.0, None, op0=OP.mult)
    UT = per.tile([R, S + 1], F32)
    nc.vector.tensor_scalar(UT[:], W['sgrid'][:], Tt[:], None, op0=OP.mult)
    B2 = per.tile([R, S], F32)
    nc.vector.tensor_tensor(B2[:], Sn[:], P2[:], op=OP.mult)
    zf = per.tile([R, S + 1], F32)
    # alternate j between a DVE-only path and a Pool-prepared path so both
    # engines chew the inverse-CDF concurrently
    for j in range(S + 1):
        if j % 2 == 0:
            x_ = pp2.tile([R, S], F32, tag="pdfx", name="x_")
            nc.vector.scalar_tensor_tensor(x_[:], Sn[:], UT[:, j:j + 1],
                                           P2[:], op0=OP.add, op1=OP.mult)
        else:
            x1 = pp2.tile([R, S], F32, tag="pdfx1", name="x1", bufs=4)
            nc.gpsimd.tensor_scalar(x1[:], P2[:], UT[:, j:j + 1], None,
                                    op0=OP.mult)
            x_ = pp2.tile([R, S], F32, tag="pdfx2", name="x_", bufs=4)
            nc.gpsimd.tensor_tensor(x_[:], x1[:], B2[:], op=OP.add)
        sc_ = pp2.tile([R, S], F32, tag="pdfsc", name="sc_", bufs=1)
        nc.vector.scalar_tensor_tensor(sc_[:], x_[:], 0.0, dzc[:],
                                       op0=OP.max, op1=OP.min,
                                       accum_out=zf[:, j:j + 1])
    nc.vector.tensor_scalar(zf[:], zf[:], zc[:, 0:1], None, op0=OP.add)
    if debug:
        nc.sync.dma_start(dbg["d_zf"][:], zf[:])
    if stage < 3:
        ctx.close()
        return

    midf = per.tile([R, S], F32)
    nc.vector.tensor_tensor(midf[:], zf[:, 0:S], zf[:, 1:S + 1], op=OP.add)
    nc.vector.tensor_scalar(midf[:], midf[:], 0.5, None, op0=OP.mult)

    # ======================= FINE PASS =======================
    sigfT = per.tile([R, S], F32, tag="sigfT")
    Rt = per.tile([R, S], F32, tag="Rt")
    Gt = per.tile([R, S], F32, tag="Gt")
    Bt = per.tile([R, S], F32, tag="Bt")
    rgb_rows = [Rt, Gt, Bt]

    for ci in range(NCHUNK):
        r0 = ci * CHUNK_RAYS
        ef = build_args(midf, r0, Bf, Cf, 106, 100, F32R)
        if debug and ci == 0:
            nc.sync.dma_start(dbg["d_arg"][:], ef[:].bitcast(F32))

        # ---- trunk, layer-major across the chunk's 4 tiles ----
        def relu_to(psum, hout_ap, bias_ap, idx):
            if idx % 2 == 0:
                nc.scalar.activation(hout_ap, psum, AF.Relu, bias=bias_ap)
            else:
                nc.vector.tensor_scalar(hout_ap, psum, bias_ap, 0.0,
                                        op0=OP.add, op1=OP.max)

        h = hp.tile([128, 2 * CN], F32R, tag="fh")
        for t in range(NTILE):
            cols = slice(t * TILE_N, (t + 1) * TILE_N)
            for m in range(2):
                ps = psA.tile([128, TILE_N], F32, tag="mmps")
                nc.tensor.matmul(ps[:], W['fW0ext'][:, m * 128:(m + 1) * 128],
                                 ef[:, cols], start=True, stop=True)
                relu_to(ps[:], h[:, m * CN + t * TILE_N:
                                 m * CN + (t + 1) * TILE_N],
                        W['fb0col'][:, m:m + 1], m)

        def lm_layer(wname, bname, hin, skip=False):
            hout = hp.tile([128, 2 * CN], F32R, tag="fh")
            for t in range(NTILE):
                cs = slice(t * TILE_N, (t + 1) * TILE_N)
                for m in range(2):
                    ps = psA.tile([128, TILE_N], F32, tag="mmps")
                    nc.tensor.matmul(ps[:], W[wname][:, m, :],
                                     hin[:, t * TILE_N:(t + 1) * TILE_N],
                                     start=True, stop=False)
                    nc.tensor.matmul(ps[:], W[wname][:, 2 + m, :],
                                     hin[:, CN + t * TILE_N:
                                          CN + (t + 1) * TILE_N],
                                     start=False, stop=not skip)
                    if skip:
                        nc.tensor.matmul(
                            ps[:], W['fWs_e_ext'][:, m * 128:(m + 1) * 128],
                            ef[:, cs], start=False, stop=True)
                    relu_to(ps[:], hout[:, m * CN + t * TILE_N:
                                        m * CN + (t + 1) * TILE_N],
                            W[bname][:, m:m + 1], m)
            return hout

        h = lm_layer('fWm0', 'fbm0col', h)
        h = lm_layer('fWm1', 'fbm1col', h)
        h = lm_layer('fWm2', 'fbm2col', h)
        h = lm_layer('fWs_h', 'fbscol', h, skip=True)
        h = lm_layer('fWp0', 'fbp0col', h)
        h = lm_layer('fWp1', 'fbp1col', h)
        h = lm_layer('fWp2', 'fbp2col', h)

        # ---- heads: sigma + view + rgb, interleaved per tile ----
        rgbS = big.tile([3, CN], F32, tag="rgbS", bufs=1)
        sb_ = dram.tile([1, CN], F32, tag="sigb")
        sigflat = pp2.tile([1, CN], F32, tag="sigflat", bufs=1)
        for t in range(NTILE):
            cols = slice(t * TILE_N, (t + 1) * TILE_N)
            gtile = ci * NTILE + t
            ps_ = psS.tile([1, TILE_N], F32, tag="sigps")
            nc.tensor.matmul(ps_[:], W['Wsig'][:, 0:1],
                             h[:, t * TILE_N:(t + 1) * TILE_N],
                             start=True, stop=False)
            nc.tensor.matmul(ps_[:], W['Wsig'][:, 1:2],
                             h[:, CN + t * TILE_N:CN + (t + 1) * TILE_N],
                             start=False, stop=True)
            nc.scalar.copy(sigflat[0:1, cols], ps_[:])

            pv = psA.tile([128, TILE_N], F32, tag="mmps")
            nc.tensor.matmul(pv[:], W['Wfc'][:, 0, :],
                             h[:, t * TILE_N:(t + 1) * TILE_N],
                             start=True, stop=False)
            nc.tensor.matmul(pv[:], W['Wfc'][:, 1, :],
                             h[:, CN + t * TILE_N:CN + (t + 1) * TILE_N],
                             start=False, stop=False)
            nc.tensor.matmul(pv[:], W['fWv_app'][:], ef[:, cols],
                             start=False, stop=False)
            nc.tensor.matmul(pv[:], hvre[:, gtile, :], W['Etile'][:],
                             start=False, stop=True)
            hv = hp.tile([128, TILE_N], F32R, tag="fhv", bufs=2)
            if t % 2 == 0:
                nc.scalar.activation(hv[:], pv[:], AF.Relu)
            else:
                nc.vector.tensor_scalar(hv[:], pv[:], 0.0, None, op0=OP.max)

            prgb = psR.tile([3, TILE_N], F32, tag="rgbps")
            nc.tensor.matmul(prgb[:], W['Wrgb'][:], hv[:],
                             start=True, stop=True)
            nc.scalar.copy(rgbS[0:3, cols], prgb[:])
        nc.sync.dma_start(sb_[:], sigflat[:])
        nc.sync.dma_start(sigfT[r0:r0 + CHUNK_RAYS, :],
                          sb_[:].rearrange("a (p f) -> (a p) f", p=CHUNK_RAYS))
        rb_ = dram.tile([3, CN], F32, tag="rgbb")
        nc.sync.dma_start(rb_[:], rgbS[:])
        for cch in range(3):
            nc.sync.dma_start(
                rgb_rows[cch][r0:r0 + CHUNK_RAYS, :],
                rb_[cch:cch + 1, :].rearrange("a (p f) -> (a p) f",
                                              p=CHUNK_RAYS))

    # ======================= tail: composite =======================
    wf, _dzf = raw2w(sigfT[:, :], zf[:, 0:S], zf[:, 1:S + 1],
                     norm[:, :], bsig_f, R, "f")
    if debug:
        nc.sync.dma_start(dbg["d_sigf"][:], sigfT[:])
        nc.sync.dma_start(dbg["d_wf"][:], wf[:])
    rgbout = per.tile([R, 3], F32)
    for cch in range(3):
        sg = per.tile([R, S], F32, tag="sg%d" % cch)
        nc.scalar.activation(sg[:], rgb_rows[cch][:], AF.Sigmoid,
                             bias=float(brgb[cch]))
        nc.vector.tensor_tensor(sg[:], sg[:], wf[:], op=OP.mult)
        nc.vector.tensor_reduce(rgbout[:, cch:cch + 1], sg[:],
                                axis=mybir.AxisListType.X, op=OP.add)
    nc.sync.dma_start(OUT[:], rgbout[:])
    ctx.close()


# ---------------------------------------------------------------- entry
_CACHE = {}


def kernel(**inputs):
    inp = {k: np.asarray(v) for k, v in inputs.items()}
    consts, scal = host_prep(inp)
    key = (BUILD_STAGE, DEBUG_OUT, scal['pbo_f'], scal['bsig_f'],
           tuple(scal['brgb']))
    if key not in _CACHE:
        _CACHE[key] = build_nc(scal['pbo_f'], scal['bsig_f'], scal['brgb'],
                               stage=BUILD_STAGE, debug=DEBUG_OUT)
    nc = _CACHE[key]
    rays = np.asarray(inp['rays'], np.float32)
    in_maps = []
    for core in range(NCORES):
        m = {k: np.ascontiguousarray(v, dtype=np.float32)
             for k, v in consts.items()}
        m['rays'] = np.ascontiguousarray(rays[core * R:(core + 1) * R])
        in_maps.append(m)
    res = run_bass_kernel_spmd(nc, in_maps, core_ids=list(range(NCORES)))
    globals()['_LAST_RESULTS'] = res
    return np.concatenate([r['rgb_out'] for r in res.results], 0)


# revision 43
# speedup vs baseline: 1.0927x; 1.0525x over previous
"""NeRF-style render kernel for TRN2 (8 NeuronCores, data-parallel over rays).

Self-contained: hardcodes all shapes. Both MLPs run in float32r (1 cycle/row
on PE for moving dim >= 256). Posenc args are built as mid*B + C rank-1
per-ray matrices pre-scaled by 1/2pi; range reduction is a 2-op round
(magic-number) + subtract, with the 2pi fold done by the activation
engine's scale parameter. Fine trunk is scheduled layer-major across the
chunk so PE pipelines without relu stalls. Exp/sigmoid are batched to
minimize activation-table reloads.
"""
import os
import sys

sys.path.insert(0, '/opt/trn_rl_repo')
import numpy as np
import concourse.bass as bass
import concourse.bacc as bacc
import concourse.tile as tile
import concourse.mybir as mybir
from concourse.bass_utils import run_bass_kernel_spmd

F32 = mybir.dt.float32
F32R = mybir.dt.float32r
AF = mybir.ActivationFunctionType
OP = mybir.AluOpType

NCORES = 8
R = 128          # rays per core
S = 128          # samples per pass
CHUNK_RAYS = 16  # rays per chunk
NCHUNK = R // CHUNK_RAYS          # 8
CN = CHUNK_RAYS * S               # 2048 cols per chunk
TILE_N = 512                      # matmul moving size
NTILE = CN // TILE_N              # 4 point-tiles per chunk

MAGIC = np.float32(12582912.0)    # 1.5 * 2^23 (round-to-int trick)
TWOPI = float(np.float32(2.0 * np.pi))
INV2PI = 1.0 / (2.0 * np.pi)      # folded into posenc matrices (fp64 host)

BUILD_STAGE = int(os.environ.get("KERNEL_STAGE", "3"))
DEBUG_OUT = os.environ.get("KERNEL_DEBUG", "0") == "1"


# ---------------------------------------------------------------- host prep
def _posenc_rows(nf, span=None, minp=None):
    """A3 [6*nf,3] / const [6*nf] for rows f-major: per f: 3 sin, 3 cos."""
    rows = 6 * nf
    A3 = np.zeros((rows, 3), np.float64)
    ph = np.zeros((rows,), np.float64)
    for f in range(nf):
        for k in range(6):
            r = 6 * f + k
            d = k % 3
            sc = 2.0 ** f
            if span is not None:
                A3[r, d] = sc / span[d]
                ph[r] = -sc * minp[d] / span[d]
            else:
                A3[r, d] = sc
            if k >= 3:
                ph[r] += np.pi / 2.0
    return A3, ph


def host_prep(inp):
    c = {}
    f32 = np.float32

    # ---- coarse: arg rows = [60 sin-args (pre /2pi), 3 raw xyz] ----
    A3s, phs = _posenc_rows(10)
    cA3 = np.concatenate([A3s * INV2PI, np.eye(3)], 0)           # [63,3]
    cph = np.concatenate([phs * INV2PI, np.zeros(3)], 0)         # [63]
    c['cA3T'] = cA3.T.astype(f32).copy()                         # [3,63]
    c['cA4T'] = np.concatenate([cA3, cph[:, None]], 1).T.astype(f32).copy()

    # ---- fine: rows [60 sinx, 4 pad, 36 sinapp, 3 xyz, 3 applin] ----
    minp = inp['min_point'].astype(np.float64)
    span = (inp['max_point'] - inp['min_point']).astype(np.float64)
    A3a, pha = _posenc_rows(6, span=span, minp=minp)
    pad4 = np.zeros((4, 3))
    fA3 = np.concatenate([A3s * INV2PI, pad4, A3a * INV2PI,
                          np.eye(3), np.diag(1.0 / span)], 0)    # [106,3]
    fph = np.concatenate([phs * INV2PI, np.zeros(4), pha * INV2PI,
                          np.zeros(3), -minp / span], 0)
    c['fA3T'] = fA3.T.astype(f32).copy()                         # [3,106]
    c['fA4T'] = np.concatenate([fA3, fph[:, None]], 1).T.astype(f32).copy()

    # per-ray enc matrices (lhsT) for viewdir/time features
    Ad = np.zeros((24, 4), np.float64)
    for f in range(4):
        for k in range(6):
            r = 6 * f + k
            Ad[r, k % 3] = 2.0 ** f
            if k >= 3:
                Ad[r, 3] = np.pi / 2.0
    c['AdT'] = Ad.T.astype(f32).copy()                           # [4,24]
    At = np.zeros((12, 2), np.float64)
    for f in range(6):
        At[2 * f, 0] = 2.0 ** f
        At[2 * f + 1, 0] = 2.0 ** f
        At[2 * f + 1, 1] = np.pi / 2.0
    c['AtT'] = At.T.astype(f32).copy()                           # [2,12]

    # coarse MLP weights: single K=63 input layer [60 sin | 3 xyz]
    c['pW0ext'] = np.concatenate([inp['pW0'][3:63], inp['pW0'][0:3]], 0).copy()
    c['pW1'] = inp['pW1'].copy()
    c['pW2'] = inp['pW2'].copy()
    c['pWo'] = inp['pWo'].copy()                                 # [128,1]
    c['pb0col'] = inp['pb0'].reshape(-1, 1).copy()
    c['pb1col'] = inp['pb1'].reshape(-1, 1).copy()
    c['pb2col'] = inp['pb2'].reshape(-1, 1).copy()

    # fine MLP weights padded to K=106 feature layout
    def ext106(Wsin60, Wlin3, width):
        out = np.zeros((106, width), f32)
        out[0:60] = Wsin60
        out[100:103] = Wlin3
        return out

    c['fW0ext'] = ext106(inp['fW0'][3:63], inp['fW0'][0:3], 256)
    c['fWs_e_ext'] = ext106(inp['fWs'][256 + 3:256 + 63],
                            inp['fWs'][256:256 + 3], 256)

    def pack_km(Wm):  # [256, 256] -> [128, 4, 128], slot 2k+m
        out = np.zeros((128, 4, 128), f32)
        for k in range(2):
            for m in range(2):
                out[:, 2 * k + m, :] = Wm[k * 128:(k + 1) * 128,
                                          m * 128:(m + 1) * 128]
        return out

    for i in range(3):
        c[f'fWm{i}'] = pack_km(inp['fWm'][i])
        c[f'fWp{i}'] = pack_km(inp['fWp'][i])
    c['fWs_h'] = pack_km(inp['fWs'][0:256])
    c['fb0col'] = inp['fb0'].reshape(2, 128).T.copy()            # [128,2]
    for i in range(3):
        c[f'fbm{i}col'] = inp['fbm'][i].reshape(2, 128).T.copy()
        c[f'fbp{i}col'] = inp['fbp'][i].reshape(2, 128).T.copy()
    c['fbscol'] = inp['fbs'].reshape(2, 128).T.copy()

    # view head: fold Wfeat into Wview
    Wv = inp['Wview']
    Wv_d, Wv_emb, Wv_t, Wv_app = (Wv[256:283], Wv[283:331],
                                  Wv[331:344], Wv[344:383])
    Wfc = (inp['Wfeat'].astype(np.float64) @ Wv[0:256].astype(np.float64)
           ).astype(f32)
    out = np.zeros((128, 2, 128), f32)
    out[:, 0, :] = Wfc[0:128]
    out[:, 1, :] = Wfc[128:256]
    c['Wfc'] = out
    c['bveffcol'] = (inp['bfeat'].astype(np.float64)
                     @ Wv[0:256].astype(np.float64)
                     + inp['bview'].astype(np.float64)
                     ).astype(f32).reshape(-1, 1)
    # app-enc weights padded to K=106 rows [64:100 sin | 103:106 linear]
    Wva = np.zeros((106, 128), f32)
    Wva[64:100] = Wv_app[3:39]
    Wva[103:106] = Wv_app[0:3]
    c['fWv_app'] = Wva
    c['Wv_d_lin'] = np.ascontiguousarray(Wv_d[0:3])
    c['Wv_d_sin'] = np.ascontiguousarray(Wv_d[3:27])
    c['Wv_emb'] = np.ascontiguousarray(Wv_emb)
    c['Wv_t_lin'] = np.ascontiguousarray(Wv_t[0:1])
    c['Wv_t_sin'] = np.ascontiguousarray(Wv_t[1:13])
    c['Wsig'] = np.stack([inp['Wsig'][0:128, 0],
                          inp['Wsig'][128:256, 0]], 1).copy()    # [128,2]
    c['Wrgb'] = inp['Wrgb'].copy()                               # [128,3]
    c['emb_table'] = inp['emb_table'].copy()

    c['sgrid'] = np.broadcast_to(
        np.arange(129, dtype=f32) / 128.0, (128, 129)).copy()
    c['identity'] = np.eye(128, dtype=f32)
    E = np.zeros((4, 512), f32)
    for rl in range(4):
        E[rl, rl * 128:(rl + 1) * 128] = 1.0
    c['Etile'] = E
    c['iotacol'] = np.arange(100, dtype=f32).reshape(-1, 1)
    scalars = dict(pbo_f=float(inp['pbo'][0]), bsig_f=float(inp['bsig'][0]),
                   brgb=[float(x) for x in inp['brgb']])
    return c, scalars


INPUT_SHAPES = {
    'rays': (R, 12),
    'cA3T': (3, 63), 'cA4T': (4, 63),
    'fA3T': (3, 106), 'fA4T': (4, 106),
    'AdT': (4, 24), 'AtT': (2, 12),
    'pW0ext': (63, 128),
    'pW1': (128, 128), 'pW2': (128, 128), 'pWo': (128, 1),
    'pb0col': (128, 1), 'pb1col': (128, 1), 'pb2col': (128, 1),
    'fW0ext': (106, 256), 'fWm0': (128, 4, 128), 'fWm1': (128, 4, 128),
    'fWm2': (128, 4, 128), 'fWp0': (128, 4, 128), 'fWp1': (128, 4, 128),
    'fWp2': (128, 4, 128), 'fWs_h': (128, 4, 128), 'fWs_e_ext': (106, 256),
    'fb0col': (128, 2), 'fbm0col': (128, 2), 'fbm1col': (128, 2),
    'fbm2col': (128, 2), 'fbp0col': (128, 2), 'fbp1col': (128, 2),
    'fbp2col': (128, 2), 'fbscol': (128, 2),
    'Wfc': (128, 2, 128), 'bveffcol': (128, 1), 'fWv_app': (106, 128),
    'Wv_d_lin': (3, 128), 'Wv_d_sin': (24, 128), 'Wv_emb': (48, 128),
    'Wv_t_lin': (1, 128), 'Wv_t_sin': (12, 128),
    'Wsig': (128, 2), 'Wrgb': (128, 3),
    'emb_table': (100, 48),
    'sgrid': (128, 129), 'identity': (128, 128),
    'Etile': (4, 512), 'iotacol': (100, 1),
}
F32R_WEIGHTS = {'fW0ext', 'fWm0', 'fWm1', 'fWm2', 'fWp0', 'fWp1', 'fWp2',
                'fWs_h', 'fWs_e_ext', 'Wfc', 'fWv_app', 'Wv_d_lin',
                'Wv_d_sin', 'Wv_emb', 'Wv_t_lin', 'Wv_t_sin', 'Wsig', 'Wrgb',
                'emb_table', 'Etile'}


# ---------------------------------------------------------------- bass build
def build_nc(pbo_f, bsig_f, brgb, stage=3, debug=False):
    nc = bacc.Bacc("TRN2", target_bir_lowering=False)
    D = {k: nc.dram_tensor(k, list(v), F32, kind="ExternalInput")
         for k, v in INPUT_SHAPES.items()}
    OUT = nc.dram_tensor("rgb_out", [R, 3], F32, kind="ExternalOutput")
    dbg = {}
    if debug:
        for nm, shp in [("d_sigc", (R, S)), ("d_zf", (R, S + 1)),
                        ("d_wc", (R, S)), ("d_sigf", (R, S)),
                        ("d_wf", (R, S)), ("d_arg", (106, CN))]:
            dbg[nm] = nc.dram_tensor(nm, list(shp), F32, kind="ExternalOutput")
    with tile.TileContext(nc) as tc:
        _body(nc, tc, D, OUT, dbg, pbo_f, bsig_f, brgb, stage, debug)
    nc.compile()
    return nc


def _body(nc, tc, D, OUT, dbg, pbo_f, bsig_f, brgb, stage, debug):
    from contextlib import ExitStack
    ctx = ExitStack()
    wpool = ctx.enter_context(tc.tile_pool(name="w", bufs=1))
    per = ctx.enter_context(tc.tile_pool(name="per", bufs=1))
    pp2 = ctx.enter_context(tc.tile_pool(name="pp2", bufs=2))
    big = ctx.enter_context(tc.tile_pool(name="big", bufs=2))
    hp = ctx.enter_context(tc.tile_pool(name="h", bufs=2))
    dram = ctx.enter_context(tc.tile_pool(name="dr", bufs=2, space="DRAM"))
    psA = ctx.enter_context(tc.tile_pool(name="psA", bufs=4, space="PSUM"))
    psS = ctx.enter_context(tc.tile_pool(name="psS", bufs=1, space="PSUM"))
    psR = ctx.enter_context(tc.tile_pool(name="psR", bufs=1, space="PSUM"))
    psC = ctx.enter_context(tc.tile_pool(name="psC", bufs=1, space="PSUM"))

    W = {}
    EARLY = ['rays', 'identity', 'sgrid', 'cA3T', 'cA4T', 'AdT', 'AtT',
             'iotacol', 'emb_table', 'Wv_d_lin', 'Wv_d_sin', 'Wv_emb',
             'Wv_t_lin', 'Wv_t_sin', 'bveffcol', 'fA3T', 'fA4T',
             'pW0ext', 'pb0col', 'pW1', 'pb1col', 'pW2', 'pb2col', 'pWo']
    order = EARLY + [k for k in D if k not in EARLY]
    for k in order:
        t = D[k]
        if k == 'rays':
            continue
        dt = F32R if k in F32R_WEIGHTS else F32
        tl = wpool.tile(list(t.shape), dt, tag="w_" + k, name="w_" + k)
        nc.sync.dma_start(tl[:], t[:].bitcast(F32R) if dt == F32R else t[:])
        W[k] = tl
    rays = wpool.tile([R, 12], F32, tag="w_rays")
    nc.sync.dma_start(rays[:], D['rays'][:])
    ident = W['identity']

    # ---------------- phase 0: per-ray prep (ray-major layout)
    nearc = per.tile([R, 1], F32)
    nc.vector.tensor_scalar(nearc[:], rays[:, 6:7], 1e-8, None, op0=OP.max)
    spanc = per.tile([R, 1], F32)
    nc.vector.tensor_tensor(spanc[:], rays[:, 7:8], nearc[:], op=OP.subtract)

    dsq = per.tile([R, 3], F32)
    nc.vector.tensor_tensor(dsq[:], rays[:, 3:6], rays[:, 3:6], op=OP.mult)
    ssum = per.tile([R, 1], F32)
    nc.vector.reduce_sum(ssum[:], dsq[:], axis=mybir.AxisListType.X)
    norm = per.tile([R, 1], F32)
    nc.scalar.activation(norm[:], ssum[:], AF.Sqrt)
    for it in range(2):
        t1 = per.tile([R, 1], F32, tag="nwt")
        nc.vector.reciprocal(t1[:], norm[:])
        nc.vector.scalar_tensor_tensor(t1[:], ssum[:], 1.0, t1[:],
                                       op0=OP.mult, op1=OP.mult)
        nc.vector.tensor_tensor(t1[:], t1[:], norm[:], op=OP.add)
        nc.vector.tensor_scalar(norm[:], t1[:], 0.5, None, op0=OP.mult)
    invn = per.tile([R, 1], F32)
    nc.vector.reciprocal(invn[:], norm[:])

    # bundle: 0:3 o, 3 ones | 4:7 dir | 8:11 viewdir, 11 ones |
    #         12 t, 13 ones, 14 embid
    bundle = per.tile([R, 20], F32)
    nc.gpsimd.memset(bundle[:], 0.0)
    nc.vector.tensor_copy(bundle[:, 0:3], rays[:, 0:3])
    nc.vector.memset(bundle[:, 3:4], 1.0)
    nc.vector.tensor_copy(bundle[:, 4:7], rays[:, 3:6])
    nc.vector.tensor_scalar(bundle[:, 8:11], rays[:, 3:6], invn[:], None,
                            op0=OP.mult)
    nc.vector.memset(bundle[:, 11:12], 1.0)
    nc.vector.tensor_copy(bundle[:, 12:13], rays[:, 8:9])
    nc.vector.memset(bundle[:, 13:14], 1.0)
    nc.vector.tensor_copy(bundle[:, 14:15], rays[:, 9:10])

    def transp(col, nm):
        p = psC.tile([4, 128], F32, tag="ptp")
        nc.tensor.transpose(p[:], bundle[:, col:col + 4], ident[:])
        sb = per.tile([4, 128], F32, tag="tp_" + nm)
        nc.scalar.copy(sb[:], p[:])
        return sb

    oT = transp(0, "o")      # [oT;ones]
    dirT = transp(4, "d")
    vdT = transp(8, "vd")    # [viewdirT;ones]
    tT = transp(12, "t")     # [t;ones;embid]
    eiT = transp(14, "ei")   # row0 = embid

    def mm_copy(lhsT, rhs, shape, nm, dst_dtype=F32):
        p = psC.tile(shape, F32, tag="pmc")
        nc.tensor.matmul(p[:], lhsT, rhs, start=True, stop=True)
        sb = per.tile(shape, dst_dtype, tag="mc_" + nm)
        nc.scalar.copy(sb[:], p[:])
        return sb

    # per-ray rank-1 posenc matrices (pre-scaled by 1/2pi on sin rows)
    Bc = mm_copy(W['cA3T'][:], dirT[0:3, :], [63, 128], "Bc")
    Cc = mm_copy(W['cA4T'][:], oT[:], [63, 128], "Cc")
    Bf = mm_copy(W['fA3T'][:], dirT[0:3, :], [106, 128], "Bf")
    Cf = mm_copy(W['fA4T'][:], oT[:], [106, 128], "Cf")

    def rangered_v(ap, shape, tag):
        sc = per.tile(shape, F32, tag=tag)
        nc.vector.tensor_scalar(sc[:], ap, float(INV2PI), float(MAGIC),
                                op0=OP.mult, op1=OP.add)
        nc.vector.tensor_scalar(sc[:], sc[:], float(MAGIC), None,
                                op0=OP.subtract)
        nc.vector.scalar_tensor_tensor(ap, sc[:], -TWOPI, ap,
                                       op0=OP.mult, op1=OP.add)

    # per-ray view features
    argd = mm_copy(W['AdT'][:], vdT[:], [24, 128], 'argd')
    rangered_v(argd[:], [24, 128], "rrd")
    sind = per.tile([24, 128], F32R)
    nc.scalar.activation(sind[:], argd[:], AF.Sin)
    vd_r = per.tile([4, 128], F32R)
    nc.vector.tensor_copy(vd_r[:], vdT[:])

    argt = mm_copy(W['AtT'][:], tT[0:2, :], [12, 128], 'argt')
    rangered_v(argt[:], [12, 128], "rrt")
    sint = per.tile([12, 128], F32R)
    nc.scalar.activation(sint[:], argt[:], AF.Sin)
    t_r = per.tile([4, 128], F32R)
    nc.vector.tensor_copy(t_r[:], tT[:])

    embBC = per.tile([100, 128], F32)
    nc.gpsimd.partition_broadcast(embBC[:], eiT[0:1, :], channels=100)
    onehot = per.tile([100, 128], F32R)
    nc.vector.tensor_scalar(onehot[:], embBC[:], W['iotacol'][:], None,
                            op0=OP.is_equal)
    embT = mm_copy(W['emb_table'][:], onehot[:], [48, 128], 'embT',
                   dst_dtype=F32R)

    phv = psC.tile([128, 128], F32, tag="pmc")
    nc.tensor.matmul(phv[:], W['Wv_d_lin'][:], vd_r[0:3, :],
                     start=True, stop=False)
    nc.tensor.matmul(phv[:], W['Wv_d_sin'][:], sind[:], start=False, stop=False)
    nc.tensor.matmul(phv[:], W['Wv_emb'][:], embT[:], start=False, stop=False)
    nc.tensor.matmul(phv[:], W['Wv_t_lin'][:], t_r[0:1, :],
                     start=False, stop=False)
    nc.tensor.matmul(phv[:], W['Wv_t_sin'][:], sint[:], start=False, stop=True)
    hvray = per.tile([128, 128], F32)
    nc.vector.tensor_scalar(hvray[:], phv[:], W['bveffcol'][:], None,
                            op0=OP.add)
    phvT = psC.tile([128, 128], F32, tag="pmc")
    nc.tensor.transpose(phvT[:], hvray[:], ident[:])
    hvrayT = per.tile([128, 128], F32R)
    nc.scalar.copy(hvrayT[:], phvT[:])
    hvb = dram.tile([128, 128], F32R, tag="hvb")
    nc.sync.dma_start(hvb[:], hvrayT[:])
    hvre = wpool.tile([4, 32, 128], F32R, tag="hvre")
    nc.sync.dma_start(hvre[:], hvb[:].rearrange("(t rl) m -> rl t m", rl=4))

    # coarse z edges
    zc = per.tile([R, S + 1], F32)
    nc.vector.tensor_scalar(zc[:], W['sgrid'][:], spanc[:], None, op0=OP.mult)
    nc.vector.tensor_scalar(zc[:], zc[:], nearc[:], None, op0=OP.add)
    midc = per.tile([R, S], F32)
    nc.vector.tensor_tensor(midc[:], zc[:, 0:S], zc[:, 1:S + 1], op=OP.add)
    nc.vector.tensor_scalar(midc[:], midc[:], 0.5, None, op0=OP.mult)

    # posenc arg builder: arg = mid*B + C; RR+sin on rows 0:nsin in place.
    def build_args(mid_src, r0, Bm, Cm, nrows, nsin, fdt):
        mbc = dram.tile([CHUNK_RAYS, S], F32, tag="midb")
        nc.sync.dma_start(mbc[:], mid_src[r0:r0 + CHUNK_RAYS, :])
        mfc = pp2.tile([1, CN], F32, tag="flat", bufs=1)
        nc.sync.dma_start(mfc[:],
                          mbc[:].rearrange("p f -> (p f)").unsqueeze(0))
        mBC = big.tile([nrows, CN], F32, tag="mbc", bufs=1)
        nc.gpsimd.partition_broadcast(mBC[:], mfc[:], channels=nrows)
        arg = big.tile([nrows, CN], F32, tag="argt", bufs=1)
        b3 = Bm[:, r0:r0 + CHUNK_RAYS].unsqueeze(2).broadcast_to(
            [nrows, CHUNK_RAYS, S])
        c3 = Cm[:, r0:r0 + CHUNK_RAYS].unsqueeze(2).broadcast_to(
            [nrows, CHUNK_RAYS, S])
        a3 = arg[:].rearrange("p (r s) -> p r s", r=CHUNK_RAYS)
        m3 = mBC[:].rearrange("p (r s) -> p r s", r=CHUNK_RAYS)
        nc.vector.tensor_tensor(a3, m3, b3, op=OP.mult)           # DVE
        nc.gpsimd.tensor_tensor(a3, a3, c3, op=OP.add)            # Pool
        # range reduction on sin rows: k = round(a); a -= k; sin(2pi*a)
        kk = big.tile([nsin, CN], F32, tag="kk", bufs=1)
        nc.vector.tensor_scalar(kk[:], arg[0:nsin, :], float(MAGIC),
                                float(MAGIC), op0=OP.add,
                                op1=OP.subtract)                  # DVE
        if fdt == F32:   # coarse: r on DVE (Pool is posenc-bound)
            nc.vector.tensor_tensor(arg[0:nsin, :], arg[0:nsin, :], kk[:],
                                    op=OP.subtract)
        else:            # fine: r on Pool (DVE is trunk-bound)
            nc.gpsimd.tensor_tensor(arg[0:nsin, :], arg[0:nsin, :], kk[:],
                                    op=OP.subtract)
        ef = big.tile([nrows, CN], fdt, tag="eft")
        nc.scalar.activation(ef[0:nsin, :], arg[0:nsin, :], AF.Sin,
                             scale=TWOPI)                         # Act
        if nrows > nsin:
            src_ap = arg[nsin:nrows, :]
            nc.sync.dma_start(ef[nsin:nrows, :],
                              src_ap.bitcast(F32R) if fdt == F32R else src_ap)
        return ef

    # ======================= raw2weights helper =======================
    def raw2w(sigT_ap, z_lo, z_hi, norm_ap, bias_f, nrows, tag):
        """w = alpha * exclusive-cumprod(1-alpha+1e-10); returns (w, dz)."""
        P = nrows
        dz = per.tile([P, S], F32, tag=tag + "dz")
        nc.vector.tensor_tensor(dz[:], z_hi, z_lo, op=OP.subtract)
        di = per.tile([P, S], F32, tag=tag + "di")
        nc.vector.tensor_scalar(di[:], dz[:], norm_ap, None, op0=OP.mult)
        s1 = per.tile([P, S], F32, tag=tag + "s1")
        nc.vector.tensor_scalar(s1[:], sigT_ap, bias_f, 0.0,
                                op0=OP.add, op1=OP.max)
        ea = per.tile([P, S], F32, tag=tag + "ea")
        nc.vector.tensor_tensor(ea[:], s1[:], di[:], op=OP.mult)
        e = per.tile([P, S], F32, tag=tag + "e")
        nc.scalar.activation(e[:], ea[:], AF.Exp, scale=-1.0)
        al = per.tile([P, S], F32, tag=tag + "al")
        nc.vector.tensor_scalar(al[:], e[:], -1.0, 1.0, op0=OP.mult, op1=OP.add)
        om = per.tile([P, S], F32, tag=tag + "om")
        nc.vector.tensor_scalar(om[:], e[:], 1e-10, None, op0=OP.add)
        tr = per.tile([P, S], F32, tag=tag + "tr")
        nc.vector.tensor_tensor_scan(tr[:], om[:], om[:], 1.0,
                                     op0=OP.mult, op1=OP.bypass)
        w = per.tile([P, S], F32, tag=tag + "w")
        nc.vector.tensor_copy(w[:, 0:1], al[:, 0:1])
        nc.vector.tensor_tensor(w[:, 1:S], al[:, 1:S], tr[:, 0:S - 1],
                                op=OP.mult)
        return w, dz

    # ================= inverse-CDF sampling (two ray-halves) ================
    # Half A is issued inside the coarse loop (overlaps coarse chunks 4-7 on
    # DVE/Pool while PE runs the MLP); half B overlaps fine chunks 0-3.
    pdf_state = {}

    def pdf_prep(hs, tag):
        H = 64
        wc, dzc = raw2w(sigcT[hs, :], zc[hs, 0:S], zc[hs, 1:S + 1],
                        norm[hs, :], pbo_f, H, "cp")
        if debug:
            nc.sync.dma_start(dbg["d_wc"][hs, :], wc[:])
        Wt = per.tile([H, S], F32, tag="pWt", name="Wt")
        nc.vector.tensor_scalar(Wt[:], wc[:], 1e-5, None, op0=OP.add)
        Sx = per.tile([H, S], F32, tag="pSx", name="Sx")
        nc.vector.memset(Sx[:, 0:1], 0.0)
        nc.vector.tensor_tensor_scan(Sx[:, 1:S], Wt[:, 0:S - 1],
                                     Wt[:, 0:S - 1], 0.0,
                                     op0=OP.add, op1=OP.bypass)
        Tt = per.tile([H, 1], F32, tag="pTt", name="Tt")
        nc.vector.tensor_tensor(Tt[:], Sx[:, S - 1:S], Wt[:, S - 1:S],
                                op=OP.add)
        P2 = per.tile([H, S], F32, tag="pP2", name="P2")
        nc.vector.reciprocal(P2[:], Wt[:])
        nc.vector.tensor_tensor(P2[:], P2[:], dzc[:], op=OP.mult)
        Sn = Sx
        nc.vector.tensor_scalar(Sn[:], Sx[:], -1.0, None, op0=OP.mult)
        UT = per.tile([H, S + 1], F32, tag="pUT", name="UT")
        nc.vector.tensor_scalar(UT[:], W['sgrid'][0:H, :], Tt[:], None,
                                op0=OP.mult)
        B2 = per.tile([H, S], F32, tag="pB2", name="B2")
        nc.vector.tensor_tensor(B2[:], Sn[:], P2[:], op=OP.mult)
        pdf_state.update(Sn=Sn, UT=UT, P2=P2, B2=B2, dzc=dzc, hs=hs)

    def pdf_js(j0, j1):
        H = 64
        Sn, UT, P2 = pdf_state['Sn'], pdf_state['UT'], pdf_state['P2']
        B2, dzc, hs = pdf_state['B2'], pdf_state['dzc'], pdf_state['hs']
        for j in range(j0, j1):
            if j % 5 < 3:
                x_ = pp2.tile([H, S], F32, tag="pdfx", name="x_")
                nc.vector.scalar_tensor_tensor(x_[:], Sn[:], UT[:, j:j + 1],
                                               P2[:], op0=OP.add, op1=OP.mult)
            else:
                x1 = pp2.tile([H, S], F32, tag="pdfx1", name="x1", bufs=4)
                nc.gpsimd.tensor_scalar(x1[:], P2[:], UT[:, j:j + 1], None,
                                        op0=OP.mult)
                x_ = pp2.tile([H, S], F32, tag="pdfx2", name="x_", bufs=4)
                nc.gpsimd.tensor_tensor(x_[:], x1[:], B2[:], op=OP.add)
            sc_ = pp2.tile([H, S], F32, tag="pdfsc", name="sc_", bufs=1)
            nc.vector.scalar_tensor_tensor(sc_[:], x_[:], 0.0, dzc[:],
                                           op0=OP.max, op1=OP.min,
                                           accum_out=zf[hs, j:j + 1])

    def pdf_finish():
        hs = pdf_state['hs']
        nc.vector.tensor_scalar(zf[hs, :], zf[hs, :], zc[hs, 0:1],
                                None, op0=OP.add)
        nc.vector.tensor_tensor(midf[hs, :], zf[hs, 0:S], zf[hs, 1:S + 1],
                                op=OP.add)
        nc.vector.tensor_scalar(midf[hs, :], midf[hs, :], 0.5, None,
                                op0=OP.mult)

    def pdf_batch_fn(step):
        if step == 0:
            pdf_prep(slice(0, 64), "A")
            pdf_js(0, 33)
        elif step < 4:
            pdf_js(33 * step, min(33 * step + 33, S + 1))
            if step == 3:
                pdf_finish()


    # ======================= COARSE PASS =======================
    sigcT = per.tile([R, S], F32, tag="sigcT")
    zf = per.tile([R, S + 1], F32)
    midf = per.tile([R, S], F32)
    pdf_batch = [pdf_batch_fn]
    for ci in range(NCHUNK):
        r0 = ci * CHUNK_RAYS
        rhs = build_args(midc, r0, Bc, Cc, 63, 60, F32)
        sb_ = dram.tile([1, CN], F32, tag="sigb")
        sigflat = pp2.tile([1, CN], F32, tag="sigflat", bufs=1)
        # layer-major across the 4 tiles so PE never stalls on relus
        ch1 = hp.tile([128, CN], F32, tag="ch1")
        for t in range(NTILE):
            cols = slice(t * TILE_N, (t + 1) * TILE_N)
            p1 = psA.tile([128, TILE_N], F32, tag="mmps")
            nc.tensor.matmul(p1[:], W['pW0ext'][:], rhs[:, cols],
                             start=True, stop=True)
            nc.scalar.activation(ch1[:, cols], p1[:], AF.Relu,
                                 bias=W['pb0col'][:])
        ch2 = hp.tile([128, CN], F32, tag="ch2", bufs=1)
        for t in range(NTILE):
            cols = slice(t * TILE_N, (t + 1) * TILE_N)
            p2 = psA.tile([128, TILE_N], F32, tag="mmps")
            nc.tensor.matmul(p2[:], W['pW1'][:], ch1[:, cols],
                             start=True, stop=True)
            nc.scalar.activation(ch2[:, cols], p2[:], AF.Relu,
                                 bias=W['pb1col'][:])
        ch3 = hp.tile([128, CN], F32, tag="ch1")
        for t in range(NTILE):
            cols = slice(t * TILE_N, (t + 1) * TILE_N)
            p3 = psA.tile([128, TILE_N], F32, tag="mmps")
            nc.tensor.matmul(p3[:], W['pW2'][:], ch2[:, cols],
                             start=True, stop=True)
            nc.scalar.activation(ch3[:, cols], p3[:], AF.Relu,
                                 bias=W['pb2col'][:])
        for t in range(NTILE):
            cols = slice(t * TILE_N, (t + 1) * TILE_N)
            ps_ = psS.tile([1, TILE_N], F32, tag="sigps")
            nc.tensor.matmul(ps_[:], W['pWo'][:], ch3[:, cols],
                             start=True, stop=True)
            nc.scalar.copy(sigflat[0:1, cols], ps_[:])
        nc.sync.dma_start(sb_[:], sigflat[:])
        nc.sync.dma_start(sigcT[r0:r0 + CHUNK_RAYS, :],
                          sb_[:].rearrange("a (p f) -> (a p) f", p=CHUNK_RAYS))
        if ci >= 3:
            pdf_batch[0](ci - 3)

    if debug:
        nc.sync.dma_start(dbg["d_sigc"][:], sigcT[:])
    if stage < 2:
        ctx.close()
        return

    pdf_prep(slice(64, 128), "B")
    pdf_js(0, S + 1)
    pdf_finish()
    if debug:
        nc.sync.dma_start(dbg["d_zf"][:], zf[:])
    if stage < 3:
        ctx.close()
        return

    # ======================= FINE PASS =======================
    sigfT = per.tile([R, S], F32, tag="sigfT")
    Rt = per.tile([R, S], F32, tag="Rt")
    Gt = per.tile([R, S], F32, tag="Gt")
    Bt = per.tile([R, S], F32, tag="Bt")
    rgb_rows = [Rt, Gt, Bt]

    for ci in range(NCHUNK):
        r0 = ci * CHUNK_RAYS
        ef = build_args(midf, r0, Bf, Cf, 106, 100, F32R)
        if debug and ci == 0:
            nc.sync.dma_start(dbg["d_arg"][:], ef[:].bitcast(F32))

        # ---- trunk, layer-major across the chunk's 4 tiles ----
        def relu_to(psum, hout_ap, bias_ap, idx):
            if idx % 2 == 0:
                nc.scalar.activation(hout_ap, psum, AF.Relu, bias=bias_ap)
            else:
                nc.vector.tensor_scalar(hout_ap, psum, bias_ap, 0.0,
                                        op0=OP.add, op1=OP.max)

        h = hp.tile([128, 2 * CN], F32R, tag="fh")
        for t in range(NTILE):
            cols = slice(t * TILE_N, (t + 1) * TILE_N)
            for m in range(2):
                ps = psA.tile([128, TILE_N], F32, tag="mmps")
                nc.tensor.matmul(ps[:], W['fW0ext'][:, m * 128:(m + 1) * 128],
                                 ef[:, cols], start=True, stop=True)
                relu_to(ps[:], h[:, m * CN + t * TILE_N:
                                 m * CN + (t + 1) * TILE_N],
                        W['fb0col'][:, m:m + 1], m)

        def lm_layer(wname, bname, hin, skip=False):
            hout = hp.tile([128, 2 * CN], F32R, tag="fh")
            for t in range(NTILE):
                cs = slice(t * TILE_N, (t + 1) * TILE_N)
                for m in range(2):
                    ps = psA.tile([128, TILE_N], F32, tag="mmps")
                    nc.tensor.matmul(ps[:], W[wname][:, m, :],
                                     hin[:, t * TILE_N:(t + 1) * TILE_N],
                                     start=True, stop=False)
                    nc.tensor.matmul(ps[:], W[wname][:, 2 + m, :],
                                     hin[:, CN + t * TILE_N:
                                          CN + (t + 1) * TILE_N],
                                     start=False, stop=not skip)
                    if skip:
                        nc.tensor.matmul(
                            ps[:], W['fWs_e_ext'][:, m * 128:(m + 1) * 128],
                            ef[:, cs], start=False, stop=True)
                    relu_to(ps[:], hout[:, m * CN + t * TILE_N:
                                        m * CN + (t + 1) * TILE_N],
                            W[bname][:, m:m + 1], m)
            return hout

        h = lm_layer('fWm0', 'fbm0col', h)
        h = lm_layer('fWm1', 'fbm1col', h)
        h = lm_layer('fWm2', 'fbm2col', h)
        h = lm_layer('fWs_h', 'fbscol', h, skip=True)
        h = lm_layer('fWp0', 'fbp0col', h)
        h = lm_layer('fWp1', 'fbp1col', h)
        h = lm_layer('fWp2', 'fbp2col', h)

        # ---- heads: sigma + view + rgb, interleaved per tile ----
        rgbS = big.tile([3, CN], F32, tag="rgbS", bufs=1)
        sb_ = dram.tile([1, CN], F32, tag="sigb")
        sigflat = pp2.tile([1, CN], F32, tag="sigflat", bufs=1)
        for t in range(NTILE):
            cols = slice(t * TILE_N, (t + 1) * TILE_N)
            gtile = ci * NTILE + t
            ps_ = psS.tile([1, TILE_N], F32, tag="sigps")
            nc.tensor.matmul(ps_[:], W['Wsig'][:, 0:1],
                             h[:, t * TILE_N:(t + 1) * TILE_N],
                             start=True, stop=False)
            nc.tensor.matmul(ps_[:], W['Wsig'][:, 1:2],
                             h[:, CN + t * TILE_N:CN + (t + 1) * TILE_N],
                             start=False, stop=True)
            nc.scalar.copy(sigflat[0:1, cols], ps_[:])

            pv = psA.tile([128, TILE_N], F32, tag="mmps")
            nc.tensor.matmul(pv[:], W['Wfc'][:, 0, :],
                             h[:, t * TILE_N:(t + 1) * TILE_N],
                             start=True, stop=False)
            nc.tensor.matmul(pv[:], W['Wfc'][:, 1, :],
                             h[:, CN + t * TILE_N:CN + (t + 1) * TILE_N],
                             start=False, stop=False)
            nc.tensor.matmul(pv[:], W['fWv_app'][:], ef[:, cols],
                             start=False, stop=False)
            nc.tensor.matmul(pv[:], hvre[:, gtile, :], W['Etile'][:],
                             start=False, stop=True)
            hv = hp.tile([128, TILE_N], F32R, tag="fhv", bufs=2)
            if t % 2 == 0:
                nc.scalar.activation(hv[:], pv[:], AF.Relu)
            else:
                nc.vector.tensor_scalar(hv[:], pv[:], 0.0, None, op0=OP.max)

            prgb = psR.tile([3, TILE_N], F32, tag="rgbps")
            nc.tensor.matmul(prgb[:], W['Wrgb'][:], hv[:],
                             start=True, stop=True)
            nc.scalar.copy(rgbS[0:3, cols], prgb[:])
        nc.sync.dma_start(sb_[:], sigflat[:])
        nc.sync.dma_start(sigfT[r0:r0 + CHUNK_RAYS, :],
                          sb_[:].rearrange("a (p f) -> (a p) f", p=CHUNK_RAYS))
        rb_ = dram.tile([3, CN], F32, tag="rgbb")
        nc.sync.dma_start(rb_[:], rgbS[:])
        for cch in range(3):
            nc.sync.dma_start(
                rgb_rows[cch][r0:r0 + CHUNK_RAYS, :],
                rb_[cch:cch + 1, :].rearrange("a (p f) -> (a p) f",
                                              p=CHUNK_RAYS))

    # ======================= tail: composite =======================
    wf, _dzf = raw2w(sigfT[:, :], zf[:, 0:S], zf[:, 1:S + 1],
                     norm[:, :], bsig_f, R, "f")
    if debug:
        nc.sync.dma_start(dbg["d_sigf"][:], sigfT[:])
        nc.sync.dma_start(dbg["d_wf"][:], wf[:])
    rgbout = per.tile([R, 3], F32)
    for cch in range(3):
        sg = per.tile([R, S], F32, tag="sg%d" % cch)
        nc.scalar.activation(sg[:], rgb_rows[cch][:], AF.Sigmoid,
                             bias=float(brgb[cch]))
        nc.vector.tensor_tensor(sg[:], sg[:], wf[:], op=OP.mult)
        nc.vector.tensor_reduce(rgbout[:, cch:cch + 1], sg[:],
                                axis=mybir.AxisListType.X, op=OP.add)
    nc.sync.dma_start(OUT[:], rgbout[:])
    ctx.close()


# ---------------------------------------------------------------- entry
_CACHE = {}


def kernel(**inputs):
    inp = {k: np.asarray(v) for k, v in inputs.items()}
    consts, scal = host_prep(inp)
    key = (BUILD_STAGE, DEBUG_OUT, scal['pbo_f'], scal['bsig_f'],
           tuple(scal['brgb']))
    if key not in _CACHE:
        _CACHE[key] = build_nc(scal['pbo_f'], scal['bsig_f'], scal['brgb'],
                               stage=BUILD_STAGE, debug=DEBUG_OUT)
    nc = _CACHE[key]
    rays = np.asarray(inp['rays'], np.float32)
    in_maps = []
    for core in range(NCORES):
        m = {k: np.ascontiguousarray(v, dtype=np.float32)
             for k, v in consts.items()}
        m['rays'] = np.ascontiguousarray(rays[core * R:(core + 1) * R])
        in_maps.append(m)
    res = run_bass_kernel_spmd(nc, in_maps, core_ids=list(range(NCORES)))
    globals()['_LAST_RESULTS'] = res
    return np.concatenate([r['rgb_out'] for r in res.results], 0)


# revision 44
# speedup vs baseline: 1.1270x; 1.0314x over previous
"""NeRF-style render kernel for TRN2 (8 NeuronCores, data-parallel over rays).

Self-contained: hardcodes all shapes. Both MLPs run in float32r (1 cycle/row
on PE for moving dim >= 256). Posenc args are built as mid*B + C rank-1
per-ray matrices pre-scaled by 1/2pi; range reduction is a 2-op round
(magic-number) + subtract, with the 2pi fold done by the activation
engine's scale parameter. Fine trunk is scheduled layer-major across the
chunk so PE pipelines without relu stalls. Exp/sigmoid are batched to
minimize activation-table reloads.
"""
import os
import sys

sys.path.insert(0, '/opt/trn_rl_repo')
import numpy as np
import concourse.bass as bass
import concourse.bacc as bacc
import concourse.tile as tile
import concourse.mybir as mybir
from concourse.bass_utils import run_bass_kernel_spmd

F32 = mybir.dt.float32
F32R = mybir.dt.float32r
AF = mybir.ActivationFunctionType
OP = mybir.AluOpType

NCORES = 8
R = 128          # rays per core
S = 128          # samples per pass
CHUNK_RAYS = 16  # rays per chunk
NCHUNK = R // CHUNK_RAYS          # 8
CN = CHUNK_RAYS * S               # 2048 cols per chunk
TILE_N = 512                      # matmul moving size
NTILE = CN // TILE_N              # 4 point-tiles per chunk

MAGIC = np.float32(12582912.0)    # 1.5 * 2^23 (round-to-int trick)
TWOPI = float(np.float32(2.0 * np.pi))
INV2PI = 1.0 / (2.0 * np.pi)      # folded into posenc matrices (fp64 host)

BUILD_STAGE = int(os.environ.get("KERNEL_STAGE", "3"))
DEBUG_OUT = os.environ.get("KERNEL_DEBUG", "0") == "1"


# ---------------------------------------------------------------- host prep
def _posenc_rows(nf, span=None, minp=None):
    """A3 [6*nf,3] / const [6*nf] for rows f-major: per f: 3 sin, 3 cos."""
    rows = 6 * nf
    A3 = np.zeros((rows, 3), np.float64)
    ph = np.zeros((rows,), np.float64)
    for f in range(nf):
        for k in range(6):
            r = 6 * f + k
            d = k % 3
            sc = 2.0 ** f
            if span is not None:
                A3[r, d] = sc / span[d]
                ph[r] = -sc * minp[d] / span[d]
            else:
                A3[r, d] = sc
            if k >= 3:
                ph[r] += np.pi / 2.0
    return A3, ph


def host_prep(inp):
    c = {}
    f32 = np.float32

    # ---- coarse: arg rows = [60 sin-args (pre /2pi), 3 raw xyz] ----
    A3s, phs = _posenc_rows(10)
    cA3 = np.concatenate([A3s * INV2PI, np.eye(3)], 0)           # [63,3]
    cph = np.concatenate([phs * INV2PI, np.zeros(3)], 0)         # [63]
    c['cA3T'] = cA3.T.astype(f32).copy()                         # [3,63]
    c['cA4T'] = np.concatenate([cA3, cph[:, None]], 1).T.astype(f32).copy()

    # ---- fine: rows [60 sinx, 4 pad, 36 sinapp, 3 xyz, 3 applin] ----
    minp = inp['min_point'].astype(np.float64)
    span = (inp['max_point'] - inp['min_point']).astype(np.float64)
    A3a, pha = _posenc_rows(6, span=span, minp=minp)
    pad4 = np.zeros((4, 3))
    fA3 = np.concatenate([A3s * INV2PI, pad4, A3a * INV2PI,
                          np.eye(3), np.diag(1.0 / span)], 0)    # [106,3]
    fph = np.concatenate([phs * INV2PI, np.zeros(4), pha * INV2PI,
                          np.zeros(3), -minp / span], 0)
    c['fA3T'] = fA3.T.astype(f32).copy()                         # [3,106]
    c['fA4T'] = np.concatenate([fA3, fph[:, None]], 1).T.astype(f32).copy()

    # per-ray enc matrices (lhsT) for viewdir/time features
    Ad = np.zeros((24, 4), np.float64)
    for f in range(4):
        for k in range(6):
            r = 6 * f + k
            Ad[r, k % 3] = 2.0 ** f
            if k >= 3:
                Ad[r, 3] = np.pi / 2.0
    c['AdT'] = Ad.T.astype(f32).copy()                           # [4,24]
    At = np.zeros((12, 2), np.float64)
    for f in range(6):
        At[2 * f, 0] = 2.0 ** f
        At[2 * f + 1, 0] = 2.0 ** f
        At[2 * f + 1, 1] = np.pi / 2.0
    c['AtT'] = At.T.astype(f32).copy()                           # [2,12]

    # coarse MLP weights: single K=63 input layer [60 sin | 3 xyz]
    c['pW0ext'] = np.concatenate([inp['pW0'][3:63], inp['pW0'][0:3]], 0).copy()
    c['pW1'] = inp['pW1'].copy()
    c['pW2'] = inp['pW2'].copy()
    c['pWo'] = inp['pWo'].copy()                                 # [128,1]
    c['pb0col'] = inp['pb0'].reshape(-1, 1).copy()
    c['pb1col'] = inp['pb1'].reshape(-1, 1).copy()
    c['pb2col'] = inp['pb2'].reshape(-1, 1).copy()

    # fine MLP weights padded to K=106 feature layout
    def ext106(Wsin60, Wlin3, width):
        out = np.zeros((106, width), f32)
        out[0:60] = Wsin60
        out[100:103] = Wlin3
        return out

    c['fW0ext'] = ext106(inp['fW0'][3:63], inp['fW0'][0:3], 256)
    c['fWs_e_ext'] = ext106(inp['fWs'][256 + 3:256 + 63],
                            inp['fWs'][256:256 + 3], 256)

    def pack_km(Wm):  # [256, 256] -> [128, 4, 128], slot 2k+m
        out = np.zeros((128, 4, 128), f32)
        for k in range(2):
            for m in range(2):
                out[:, 2 * k + m, :] = Wm[k * 128:(k + 1) * 128,
                                          m * 128:(m + 1) * 128]
        return out

    for i in range(3):
        c[f'fWm{i}'] = pack_km(inp['fWm'][i])
        c[f'fWp{i}'] = pack_km(inp['fWp'][i])
    c['fWs_h'] = pack_km(inp['fWs'][0:256])
    c['fb0col'] = inp['fb0'].reshape(2, 128).T.copy()            # [128,2]
    for i in range(3):
        c[f'fbm{i}col'] = inp['fbm'][i].reshape(2, 128).T.copy()
        c[f'fbp{i}col'] = inp['fbp'][i].reshape(2, 128).T.copy()
    c['fbscol'] = inp['fbs'].reshape(2, 128).T.copy()

    # view head: fold Wfeat into Wview
    Wv = inp['Wview']
    Wv_d, Wv_emb, Wv_t, Wv_app = (Wv[256:283], Wv[283:331],
                                  Wv[331:344], Wv[344:383])
    Wfc = (inp['Wfeat'].astype(np.float64) @ Wv[0:256].astype(np.float64)
           ).astype(f32)
    out = np.zeros((128, 2, 128), f32)
    out[:, 0, :] = Wfc[0:128]
    out[:, 1, :] = Wfc[128:256]
    c['Wfc'] = out
    c['bveffcol'] = (inp['bfeat'].astype(np.float64)
                     @ Wv[0:256].astype(np.float64)
                     + inp['bview'].astype(np.float64)
                     ).astype(f32).reshape(-1, 1)
    # app-enc weights padded to K=106 rows [64:100 sin | 103:106 linear]
    Wva = np.zeros((106, 128), f32)
    Wva[64:100] = Wv_app[3:39]
    Wva[103:106] = Wv_app[0:3]
    c['fWv_app'] = Wva
    c['Wv_d_lin'] = np.ascontiguousarray(Wv_d[0:3])
    c['Wv_d_sin'] = np.ascontiguousarray(Wv_d[3:27])
    c['Wv_emb'] = np.ascontiguousarray(Wv_emb)
    c['Wv_t_lin'] = np.ascontiguousarray(Wv_t[0:1])
    c['Wv_t_sin'] = np.ascontiguousarray(Wv_t[1:13])
    c['Wsig'] = np.stack([inp['Wsig'][0:128, 0],
                          inp['Wsig'][128:256, 0]], 1).copy()    # [128,2]
    c['Wrgb'] = inp['Wrgb'].copy()                               # [128,3]
    c['emb_table'] = inp['emb_table'].copy()

    c['sgrid'] = np.broadcast_to(
        np.arange(129, dtype=f32) / 128.0, (128, 129)).copy()
    c['identity'] = np.eye(128, dtype=f32)
    E = np.zeros((4, 512), f32)
    for rl in range(4):
        E[rl, rl * 128:(rl + 1) * 128] = 1.0
    c['Etile'] = E
    c['iotacol'] = np.arange(100, dtype=f32).reshape(-1, 1)
    scalars = dict(pbo_f=float(inp['pbo'][0]), bsig_f=float(inp['bsig'][0]),
                   brgb=[float(x) for x in inp['brgb']])
    return c, scalars


INPUT_SHAPES = {
    'rays': (R, 12),
    'cA3T': (3, 63), 'cA4T': (4, 63),
    'fA3T': (3, 106), 'fA4T': (4, 106),
    'AdT': (4, 24), 'AtT': (2, 12),
    'pW0ext': (63, 128),
    'pW1': (128, 128), 'pW2': (128, 128), 'pWo': (128, 1),
    'pb0col': (128, 1), 'pb1col': (128, 1), 'pb2col': (128, 1),
    'fW0ext': (106, 256), 'fWm0': (128, 4, 128), 'fWm1': (128, 4, 128),
    'fWm2': (128, 4, 128), 'fWp0': (128, 4, 128), 'fWp1': (128, 4, 128),
    'fWp2': (128, 4, 128), 'fWs_h': (128, 4, 128), 'fWs_e_ext': (106, 256),
    'fb0col': (128, 2), 'fbm0col': (128, 2), 'fbm1col': (128, 2),
    'fbm2col': (128, 2), 'fbp0col': (128, 2), 'fbp1col': (128, 2),
    'fbp2col': (128, 2), 'fbscol': (128, 2),
    'Wfc': (128, 2, 128), 'bveffcol': (128, 1), 'fWv_app': (106, 128),
    'Wv_d_lin': (3, 128), 'Wv_d_sin': (24, 128), 'Wv_emb': (48, 128),
    'Wv_t_lin': (1, 128), 'Wv_t_sin': (12, 128),
    'Wsig': (128, 2), 'Wrgb': (128, 3),
    'emb_table': (100, 48),
    'sgrid': (128, 129), 'identity': (128, 128),
    'Etile': (4, 512), 'iotacol': (100, 1),
}
F32R_WEIGHTS = {'fW0ext', 'fWm0', 'fWm1', 'fWm2', 'fWp0', 'fWp1', 'fWp2',
                'fWs_h', 'fWs_e_ext', 'Wfc', 'fWv_app', 'Wv_d_lin',
                'Wv_d_sin', 'Wv_emb', 'Wv_t_lin', 'Wv_t_sin', 'Wsig', 'Wrgb',
                'emb_table', 'Etile'}


# ---------------------------------------------------------------- bass build
def build_nc(pbo_f, bsig_f, brgb, stage=3, debug=False):
    nc = bacc.Bacc("TRN2", target_bir_lowering=False)
    D = {k: nc.dram_tensor(k, list(v), F32, kind="ExternalInput")
         for k, v in INPUT_SHAPES.items()}
    OUT = nc.dram_tensor("rgb_out", [R, 3], F32, kind="ExternalOutput")
    dbg = {}
    if debug:
        for nm, shp in [("d_sigc", (R, S)), ("d_zf", (R, S + 1)),
                        ("d_wc", (R, S)), ("d_sigf", (R, S)),
                        ("d_wf", (R, S)), ("d_arg", (106, CN))]:
            dbg[nm] = nc.dram_tensor(nm, list(shp), F32, kind="ExternalOutput")
    with tile.TileContext(nc) as tc:
        _body(nc, tc, D, OUT, dbg, pbo_f, bsig_f, brgb, stage, debug)
    nc.compile()
    return nc


def _body(nc, tc, D, OUT, dbg, pbo_f, bsig_f, brgb, stage, debug):
    from contextlib import ExitStack
    ctx = ExitStack()
    wpool = ctx.enter_context(tc.tile_pool(name="w", bufs=1))
    per = ctx.enter_context(tc.tile_pool(name="per", bufs=1))
    pp2 = ctx.enter_context(tc.tile_pool(name="pp2", bufs=2))
    big = ctx.enter_context(tc.tile_pool(name="big", bufs=2))
    hp = ctx.enter_context(tc.tile_pool(name="h", bufs=2))
    dram = ctx.enter_context(tc.tile_pool(name="dr", bufs=2, space="DRAM"))
    psA = ctx.enter_context(tc.tile_pool(name="psA", bufs=4, space="PSUM"))
    psS = ctx.enter_context(tc.tile_pool(name="psS", bufs=1, space="PSUM"))
    psR = ctx.enter_context(tc.tile_pool(name="psR", bufs=1, space="PSUM"))
    psC = ctx.enter_context(tc.tile_pool(name="psC", bufs=1, space="PSUM"))

    W = {}
    EARLY = ['rays', 'identity', 'sgrid', 'cA3T', 'cA4T', 'AdT', 'AtT',
             'iotacol', 'emb_table', 'Wv_d_lin', 'Wv_d_sin', 'Wv_emb',
             'Wv_t_lin', 'Wv_t_sin', 'bveffcol', 'fA3T', 'fA4T',
             'pW0ext', 'pb0col', 'pW1', 'pb1col', 'pW2', 'pb2col', 'pWo']
    order = EARLY + [k for k in D if k not in EARLY]
    for k in order:
        t = D[k]
        if k == 'rays':
            continue
        dt = F32R if k in F32R_WEIGHTS else F32
        tl = wpool.tile(list(t.shape), dt, tag="w_" + k, name="w_" + k)
        nc.sync.dma_start(tl[:], t[:].bitcast(F32R) if dt == F32R else t[:])
        W[k] = tl
    rays = wpool.tile([R, 12], F32, tag="w_rays")
    nc.sync.dma_start(rays[:], D['rays'][:])
    ident = W['identity']

    # ---------------- phase 0: per-ray prep (ray-major layout)
    nearc = per.tile([R, 1], F32)
    nc.vector.tensor_scalar(nearc[:], rays[:, 6:7], 1e-8, None, op0=OP.max)
    spanc = per.tile([R, 1], F32)
    nc.vector.tensor_tensor(spanc[:], rays[:, 7:8], nearc[:], op=OP.subtract)

    dsq = per.tile([R, 3], F32)
    nc.vector.tensor_tensor(dsq[:], rays[:, 3:6], rays[:, 3:6], op=OP.mult)
    ssum = per.tile([R, 1], F32)
    nc.vector.reduce_sum(ssum[:], dsq[:], axis=mybir.AxisListType.X)
    norm = per.tile([R, 1], F32)
    nc.scalar.activation(norm[:], ssum[:], AF.Sqrt)
    for it in range(2):
        t1 = per.tile([R, 1], F32, tag="nwt")
        nc.vector.reciprocal(t1[:], norm[:])
        nc.vector.scalar_tensor_tensor(t1[:], ssum[:], 1.0, t1[:],
                                       op0=OP.mult, op1=OP.mult)
        nc.vector.tensor_tensor(t1[:], t1[:], norm[:], op=OP.add)
        nc.vector.tensor_scalar(norm[:], t1[:], 0.5, None, op0=OP.mult)
    invn = per.tile([R, 1], F32)
    nc.vector.reciprocal(invn[:], norm[:])

    # bundle: 0:3 o, 3 ones | 4:7 dir | 8:11 viewdir, 11 ones |
    #         12 t, 13 ones, 14 embid
    bundle = per.tile([R, 20], F32)
    nc.gpsimd.memset(bundle[:], 0.0)
    nc.vector.tensor_copy(bundle[:, 0:3], rays[:, 0:3])
    nc.vector.memset(bundle[:, 3:4], 1.0)
    nc.vector.tensor_copy(bundle[:, 4:7], rays[:, 3:6])
    nc.vector.tensor_scalar(bundle[:, 8:11], rays[:, 3:6], invn[:], None,
                            op0=OP.mult)
    nc.vector.memset(bundle[:, 11:12], 1.0)
    nc.vector.tensor_copy(bundle[:, 12:13], rays[:, 8:9])
    nc.vector.memset(bundle[:, 13:14], 1.0)
    nc.vector.tensor_copy(bundle[:, 14:15], rays[:, 9:10])

    def transp(col, nm):
        p = psC.tile([4, 128], F32, tag="ptp")
        nc.tensor.transpose(p[:], bundle[:, col:col + 4], ident[:])
        sb = per.tile([4, 128], F32, tag="tp_" + nm)
        nc.scalar.copy(sb[:], p[:])
        return sb

    oT = transp(0, "o")      # [oT;ones]
    dirT = transp(4, "d")
    vdT = transp(8, "vd")    # [viewdirT;ones]
    tT = transp(12, "t")     # [t;ones;embid]
    eiT = transp(14, "ei")   # row0 = embid

    def mm_copy(lhsT, rhs, shape, nm, dst_dtype=F32):
        p = psC.tile(shape, F32, tag="pmc")
        nc.tensor.matmul(p[:], lhsT, rhs, start=True, stop=True)
        sb = per.tile(shape, dst_dtype, tag="mc_" + nm)
        nc.scalar.copy(sb[:], p[:])
        return sb

    # per-ray rank-1 posenc matrices (pre-scaled by 1/2pi on sin rows)
    Bc = mm_copy(W['cA3T'][:], dirT[0:3, :], [63, 128], "Bc")
    Cc = mm_copy(W['cA4T'][:], oT[:], [63, 128], "Cc")
    Bf = mm_copy(W['fA3T'][:], dirT[0:3, :], [106, 128], "Bf")
    Cf = mm_copy(W['fA4T'][:], oT[:], [106, 128], "Cf")

    def rangered_v(ap, shape, tag):
        sc = per.tile(shape, F32, tag=tag)
        nc.vector.tensor_scalar(sc[:], ap, float(INV2PI), float(MAGIC),
                                op0=OP.mult, op1=OP.add)
        nc.vector.tensor_scalar(sc[:], sc[:], float(MAGIC), None,
                                op0=OP.subtract)
        nc.vector.scalar_tensor_tensor(ap, sc[:], -TWOPI, ap,
                                       op0=OP.mult, op1=OP.add)

    # per-ray view features
    argd = mm_copy(W['AdT'][:], vdT[:], [24, 128], 'argd')
    rangered_v(argd[:], [24, 128], "rrd")
    sind = per.tile([24, 128], F32R)
    nc.scalar.activation(sind[:], argd[:], AF.Sin)
    vd_r = per.tile([4, 128], F32R)
    nc.vector.tensor_copy(vd_r[:], vdT[:])

    argt = mm_copy(W['AtT'][:], tT[0:2, :], [12, 128], 'argt')
    rangered_v(argt[:], [12, 128], "rrt")
    sint = per.tile([12, 128], F32R)
    nc.scalar.activation(sint[:], argt[:], AF.Sin)
    t_r = per.tile([4, 128], F32R)
    nc.vector.tensor_copy(t_r[:], tT[:])

    embBC = per.tile([100, 128], F32)
    nc.gpsimd.partition_broadcast(embBC[:], eiT[0:1, :], channels=100)
    onehot = per.tile([100, 128], F32R)
    nc.vector.tensor_scalar(onehot[:], embBC[:], W['iotacol'][:], None,
                            op0=OP.is_equal)
    embT = mm_copy(W['emb_table'][:], onehot[:], [48, 128], 'embT',
                   dst_dtype=F32R)

    phv = psC.tile([128, 128], F32, tag="pmc")
    nc.tensor.matmul(phv[:], W['Wv_d_lin'][:], vd_r[0:3, :],
                     start=True, stop=False)
    nc.tensor.matmul(phv[:], W['Wv_d_sin'][:], sind[:], start=False, stop=False)
    nc.tensor.matmul(phv[:], W['Wv_emb'][:], embT[:], start=False, stop=False)
    nc.tensor.matmul(phv[:], W['Wv_t_lin'][:], t_r[0:1, :],
                     start=False, stop=False)
    nc.tensor.matmul(phv[:], W['Wv_t_sin'][:], sint[:], start=False, stop=True)
    hvray = per.tile([128, 128], F32)
    nc.vector.tensor_scalar(hvray[:], phv[:], W['bveffcol'][:], None,
                            op0=OP.add)
    phvT = psC.tile([128, 128], F32, tag="pmc")
    nc.tensor.transpose(phvT[:], hvray[:], ident[:])
    hvrayT = per.tile([128, 128], F32R)
    nc.scalar.copy(hvrayT[:], phvT[:])
    hvb = dram.tile([128, 128], F32R, tag="hvb")
    nc.sync.dma_start(hvb[:], hvrayT[:])
    hvre = wpool.tile([4, 32, 128], F32R, tag="hvre")
    nc.sync.dma_start(hvre[:], hvb[:].rearrange("(t rl) m -> rl t m", rl=4))

    # coarse z edges
    zc = per.tile([R, S + 1], F32)
    nc.vector.tensor_scalar(zc[:], W['sgrid'][:], spanc[:], None, op0=OP.mult)
    nc.vector.tensor_scalar(zc[:], zc[:], nearc[:], None, op0=OP.add)
    midc = per.tile([R, S], F32)
    nc.vector.tensor_tensor(midc[:], zc[:, 0:S], zc[:, 1:S + 1], op=OP.add)
    nc.vector.tensor_scalar(midc[:], midc[:], 0.5, None, op0=OP.mult)

    # posenc arg builder: arg = mid*B + C; RR+sin on rows 0:nsin in place.
    def build_args(mid_src, r0, Bm, Cm, nrows, nsin, fdt):
        mbc = dram.tile([CHUNK_RAYS, S], F32, tag="midb")
        nc.sync.dma_start(mbc[:], mid_src[r0:r0 + CHUNK_RAYS, :])
        mfc = pp2.tile([1, CN], F32, tag="flat", bufs=1)
        nc.sync.dma_start(mfc[:],
                          mbc[:].rearrange("p f -> (p f)").unsqueeze(0))
        mBC = big.tile([nrows, CN], F32, tag="mbc", bufs=1)
        nc.gpsimd.partition_broadcast(mBC[:], mfc[:], channels=nrows)
        arg = big.tile([nrows, CN], F32, tag="argt", bufs=1)
        b3 = Bm[:, r0:r0 + CHUNK_RAYS].unsqueeze(2).broadcast_to(
            [nrows, CHUNK_RAYS, S])
        c3 = Cm[:, r0:r0 + CHUNK_RAYS].unsqueeze(2).broadcast_to(
            [nrows, CHUNK_RAYS, S])
        a3 = arg[:].rearrange("p (r s) -> p r s", r=CHUNK_RAYS)
        m3 = mBC[:].rearrange("p (r s) -> p r s", r=CHUNK_RAYS)
        nc.vector.tensor_tensor(a3, m3, b3, op=OP.mult)           # DVE
        nc.gpsimd.tensor_tensor(a3, a3, c3, op=OP.add)            # Pool
        # range reduction on sin rows: k = round(a); a -= k; sin(2pi*a)
        kk = big.tile([nsin, CN], F32, tag="kk", bufs=1)
        nc.vector.tensor_scalar(kk[:], arg[0:nsin, :], float(MAGIC),
                                float(MAGIC), op0=OP.add,
                                op1=OP.subtract)                  # DVE
        if fdt == F32:   # coarse: r on DVE (Pool is posenc-bound)
            nc.vector.tensor_tensor(arg[0:nsin, :], arg[0:nsin, :], kk[:],
                                    op=OP.subtract)
        else:            # fine: r on Pool (DVE is trunk-bound)
            nc.gpsimd.tensor_tensor(arg[0:nsin, :], arg[0:nsin, :], kk[:],
                                    op=OP.subtract)
        ef = big.tile([nrows, CN], fdt, tag="eft")
        nc.scalar.activation(ef[0:nsin, :], arg[0:nsin, :], AF.Sin,
                             scale=TWOPI)                         # Act
        if nrows > nsin:
            src_ap = arg[nsin:nrows, :]
            nc.sync.dma_start(ef[nsin:nrows, :],
                              src_ap.bitcast(F32R) if fdt == F32R else src_ap)
        return ef

    # ======================= raw2weights helper =======================
    def raw2w(sigT_ap, z_lo, z_hi, norm_ap, bias_f, nrows, tag):
        """w = alpha * exclusive-cumprod(1-alpha+1e-10); returns (w, dz)."""
        P = nrows
        dz = per.tile([P, S], F32, tag=tag + "dz")
        nc.vector.tensor_tensor(dz[:], z_hi, z_lo, op=OP.subtract)
        di = per.tile([P, S], F32, tag=tag + "di")
        nc.vector.tensor_scalar(di[:], dz[:], norm_ap, None, op0=OP.mult)
        s1 = per.tile([P, S], F32, tag=tag + "s1")
        nc.vector.tensor_scalar(s1[:], sigT_ap, bias_f, 0.0,
                                op0=OP.add, op1=OP.max)
        ea = per.tile([P, S], F32, tag=tag + "ea")
        nc.vector.tensor_tensor(ea[:], s1[:], di[:], op=OP.mult)
        e = per.tile([P, S], F32, tag=tag + "e")
        nc.scalar.activation(e[:], ea[:], AF.Exp, scale=-1.0)
        al = per.tile([P, S], F32, tag=tag + "al")
        nc.vector.tensor_scalar(al[:], e[:], -1.0, 1.0, op0=OP.mult, op1=OP.add)
        om = per.tile([P, S], F32, tag=tag + "om")
        nc.vector.tensor_scalar(om[:], e[:], 1e-10, None, op0=OP.add)
        tr = per.tile([P, S], F32, tag=tag + "tr")
        nc.vector.tensor_tensor_scan(tr[:], om[:], om[:], 1.0,
                                     op0=OP.mult, op1=OP.bypass)
        w = per.tile([P, S], F32, tag=tag + "w")
        nc.vector.tensor_copy(w[:, 0:1], al[:, 0:1])
        nc.vector.tensor_tensor(w[:, 1:S], al[:, 1:S], tr[:, 0:S - 1],
                                op=OP.mult)
        return w, dz

    # ================= inverse-CDF sampling (two ray-halves) ================
    # Half A is issued inside the coarse loop (overlaps coarse chunks 4-7 on
    # DVE/Pool while PE runs the MLP); half B overlaps fine chunks 0-3.
    pdf_state = {}

    def pdf_prep(hs, tag):
        H = 64
        wc, dzc = raw2w(sigcT[hs, :], zc[hs, 0:S], zc[hs, 1:S + 1],
                        norm[hs, :], pbo_f, H, "cp")
        if debug:
            nc.sync.dma_start(dbg["d_wc"][hs, :], wc[:])
        Wt = per.tile([H, S], F32, tag="pWt", name="Wt")
        nc.vector.tensor_scalar(Wt[:], wc[:], 1e-5, None, op0=OP.add)
        Sx = per.tile([H, S], F32, tag="pSx", name="Sx")
        nc.vector.memset(Sx[:, 0:1], 0.0)
        nc.vector.tensor_tensor_scan(Sx[:, 1:S], Wt[:, 0:S - 1],
                                     Wt[:, 0:S - 1], 0.0,
                                     op0=OP.add, op1=OP.bypass)
        Tt = per.tile([H, 1], F32, tag="pTt", name="Tt")
        nc.vector.tensor_tensor(Tt[:], Sx[:, S - 1:S], Wt[:, S - 1:S],
                                op=OP.add)
        P2 = per.tile([H, S], F32, tag="pP2", name="P2")
        nc.vector.reciprocal(P2[:], Wt[:])
        nc.vector.tensor_tensor(P2[:], P2[:], dzc[:], op=OP.mult)
        Sn = Sx
        nc.vector.tensor_scalar(Sn[:], Sx[:], -1.0, None, op0=OP.mult)
        UT = per.tile([H, S + 1], F32, tag="pUT", name="UT")
        nc.vector.tensor_scalar(UT[:], W['sgrid'][0:H, :], Tt[:], None,
                                op0=OP.mult)
        B2 = per.tile([H, S], F32, tag="pB2", name="B2")
        nc.vector.tensor_tensor(B2[:], Sn[:], P2[:], op=OP.mult)
        pdf_state.update(Sn=Sn, UT=UT, P2=P2, B2=B2, dzc=dzc, hs=hs)

    def pdf_js(j0, j1):
        H = 64
        Sn, UT, P2 = pdf_state['Sn'], pdf_state['UT'], pdf_state['P2']
        B2, dzc, hs = pdf_state['B2'], pdf_state['dzc'], pdf_state['hs']
        for j in range(j0, j1):
            if j % 5 < 3:
                x_ = pp2.tile([H, S], F32, tag="pdfx", name="x_")
                nc.vector.scalar_tensor_tensor(x_[:], Sn[:], UT[:, j:j + 1],
                                               P2[:], op0=OP.add, op1=OP.mult)
            else:
                x1 = pp2.tile([H, S], F32, tag="pdfx1", name="x1", bufs=4)
                nc.gpsimd.tensor_scalar(x1[:], P2[:], UT[:, j:j + 1], None,
                                        op0=OP.mult)
                x_ = pp2.tile([H, S], F32, tag="pdfx2", name="x_", bufs=4)
                nc.gpsimd.tensor_tensor(x_[:], x1[:], B2[:], op=OP.add)
            sc_ = pp2.tile([H, S], F32, tag="pdfsc", name="sc_", bufs=1)
            nc.vector.scalar_tensor_tensor(sc_[:], x_[:], 0.0, dzc[:],
                                           op0=OP.max, op1=OP.min,
                                           accum_out=zf[hs, j:j + 1])

    def pdf_finish():
        hs = pdf_state['hs']
        nc.vector.tensor_scalar(zf[hs, :], zf[hs, :], zc[hs, 0:1],
                                None, op0=OP.add)
        nc.vector.tensor_tensor(midf[hs, :], zf[hs, 0:S], zf[hs, 1:S + 1],
                                op=OP.add)
        nc.vector.tensor_scalar(midf[hs, :], midf[hs, :], 0.5, None,
                                op0=OP.mult)

    def pdf_batch_fn(step):
        if step == 0:
            pdf_prep(slice(0, 64), "A")
            pdf_js(0, 33)
        elif step < 4:
            pdf_js(33 * step, min(33 * step + 33, S + 1))
            if step == 3:
                pdf_finish()


    # ======================= COARSE PASS =======================
    sigcT = per.tile([R, S], F32, tag="sigcT")
    zf = per.tile([R, S + 1], F32)
    midf = per.tile([R, S], F32)
    pdf_batch = [pdf_batch_fn]
    for ci in range(NCHUNK):
        r0 = ci * CHUNK_RAYS
        rhs = build_args(midc, r0, Bc, Cc, 63, 60, F32)
        sb_ = dram.tile([1, CN], F32, tag="sigb")
        sigflat = pp2.tile([1, CN], F32, tag="sigflat", bufs=1)
        # layer-major across the 4 tiles so PE never stalls on relus
        ch1 = hp.tile([128, CN], F32, tag="ch1")
        for t in range(NTILE):
            cols = slice(t * TILE_N, (t + 1) * TILE_N)
            p1 = psA.tile([128, TILE_N], F32, tag="mmps")
            nc.tensor.matmul(p1[:], W['pW0ext'][:], rhs[:, cols],
                             start=True, stop=True)
            nc.scalar.activation(ch1[:, cols], p1[:], AF.Relu,
                                 bias=W['pb0col'][:])
        ch2 = hp.tile([128, CN], F32, tag="ch2", bufs=1)
        for t in range(NTILE):
            cols = slice(t * TILE_N, (t + 1) * TILE_N)
            p2 = psA.tile([128, TILE_N], F32, tag="mmps")
            nc.tensor.matmul(p2[:], W['pW1'][:], ch1[:, cols],
                             start=True, stop=True)
            nc.scalar.activation(ch2[:, cols], p2[:], AF.Relu,
                                 bias=W['pb1col'][:])
        ch3 = hp.tile([128, CN], F32, tag="ch1")
        for t in range(NTILE):
            cols = slice(t * TILE_N, (t + 1) * TILE_N)
            p3 = psA.tile([128, TILE_N], F32, tag="mmps")
            nc.tensor.matmul(p3[:], W['pW2'][:], ch2[:, cols],
                             start=True, stop=True)
            nc.scalar.activation(ch3[:, cols], p3[:], AF.Relu,
                                 bias=W['pb2col'][:])
        for t in range(NTILE):
            cols = slice(t * TILE_N, (t + 1) * TILE_N)
            ps_ = psS.tile([1, TILE_N], F32, tag="sigps")
            nc.tensor.matmul(ps_[:], W['pWo'][:], ch3[:, cols],
                             start=True, stop=True)
            nc.scalar.copy(sigflat[0:1, cols], ps_[:])
        nc.sync.dma_start(sb_[:], sigflat[:])
        nc.sync.dma_start(sigcT[r0:r0 + CHUNK_RAYS, :],
                          sb_[:].rearrange("a (p f) -> (a p) f", p=CHUNK_RAYS))
        if ci >= 3:
            pdf_batch[0](ci - 3)

    if debug:
        nc.sync.dma_start(dbg["d_sigc"][:], sigcT[:])
    if stage < 2:
        ctx.close()
        return

    def pdf_batchB_fn(step):
        if step == 0:
            pdf_prep(slice(64, 128), "B")
            pdf_js(0, 33)
        elif step < 4:
            pdf_js(33 * step, min(33 * step + 33, S + 1))
            if step == 3:
                pdf_finish()
    if debug:
        nc.sync.dma_start(dbg["d_zf"][:], zf[:])
    if stage < 3:
        ctx.close()
        return

    # ======================= FINE PASS =======================
    sigfT = per.tile([R, S], F32, tag="sigfT")
    Rt = per.tile([R, S], F32, tag="Rt")
    Gt = per.tile([R, S], F32, tag="Gt")
    Bt = per.tile([R, S], F32, tag="Bt")
    rgb_rows = [Rt, Gt, Bt]

    for ci in range(NCHUNK):
        r0 = ci * CHUNK_RAYS
        ef = build_args(midf, r0, Bf, Cf, 106, 100, F32R)
        if debug and ci == 0:
            nc.sync.dma_start(dbg["d_arg"][:], ef[:].bitcast(F32))

        # ---- trunk, layer-major across the chunk's 4 tiles ----
        def relu_to(psum, hout_ap, bias_ap, idx):
            if idx % 2 == 0:
                nc.scalar.activation(hout_ap, psum, AF.Relu, bias=bias_ap)
            else:
                nc.vector.tensor_scalar(hout_ap, psum, bias_ap, 0.0,
                                        op0=OP.add, op1=OP.max)

        h = hp.tile([128, 2 * CN], F32R, tag="fh")
        for t in range(NTILE):
            cols = slice(t * TILE_N, (t + 1) * TILE_N)
            for m in range(2):
                ps = psA.tile([128, TILE_N], F32, tag="mmps")
                nc.tensor.matmul(ps[:], W['fW0ext'][:, m * 128:(m + 1) * 128],
                                 ef[:, cols], start=True, stop=True)
                relu_to(ps[:], h[:, m * CN + t * TILE_N:
                                 m * CN + (t + 1) * TILE_N],
                        W['fb0col'][:, m:m + 1], m)

        def lm_layer(wname, bname, hin, skip=False):
            hout = hp.tile([128, 2 * CN], F32R, tag="fh")
            for t in range(NTILE):
                cs = slice(t * TILE_N, (t + 1) * TILE_N)
                for m in range(2):
                    ps = psA.tile([128, TILE_N], F32, tag="mmps")
                    nc.tensor.matmul(ps[:], W[wname][:, m, :],
                                     hin[:, t * TILE_N:(t + 1) * TILE_N],
                                     start=True, stop=False)
                    nc.tensor.matmul(ps[:], W[wname][:, 2 + m, :],
                                     hin[:, CN + t * TILE_N:
                                          CN + (t + 1) * TILE_N],
                                     start=False, stop=not skip)
                    if skip:
                        nc.tensor.matmul(
                            ps[:], W['fWs_e_ext'][:, m * 128:(m + 1) * 128],
                            ef[:, cs], start=False, stop=True)
                    relu_to(ps[:], hout[:, m * CN + t * TILE_N:
                                        m * CN + (t + 1) * TILE_N],
                            W[bname][:, m:m + 1], m)
            return hout

        h = lm_layer('fWm0', 'fbm0col', h)
        h = lm_layer('fWm1', 'fbm1col', h)
        h = lm_layer('fWm2', 'fbm2col', h)
        h = lm_layer('fWs_h', 'fbscol', h, skip=True)
        h = lm_layer('fWp0', 'fbp0col', h)
        h = lm_layer('fWp1', 'fbp1col', h)
        h = lm_layer('fWp2', 'fbp2col', h)

        # ---- heads: sigma + view + rgb, interleaved per tile ----
        rgbS = big.tile([3, CN], F32, tag="rgbS", bufs=1)
        sb_ = dram.tile([1, CN], F32, tag="sigb")
        sigflat = pp2.tile([1, CN], F32, tag="sigflat", bufs=1)
        for t in range(NTILE):
            cols = slice(t * TILE_N, (t + 1) * TILE_N)
            gtile = ci * NTILE + t
            ps_ = psS.tile([1, TILE_N], F32, tag="sigps")
            nc.tensor.matmul(ps_[:], W['Wsig'][:, 0:1],
                             h[:, t * TILE_N:(t + 1) * TILE_N],
                             start=True, stop=False)
            nc.tensor.matmul(ps_[:], W['Wsig'][:, 1:2],
                             h[:, CN + t * TILE_N:CN + (t + 1) * TILE_N],
                             start=False, stop=True)
            nc.scalar.copy(sigflat[0:1, cols], ps_[:])

            pv = psA.tile([128, TILE_N], F32, tag="mmps")
            nc.tensor.matmul(pv[:], W['Wfc'][:, 0, :],
                             h[:, t * TILE_N:(t + 1) * TILE_N],
                             start=True, stop=False)
            nc.tensor.matmul(pv[:], W['Wfc'][:, 1, :],
                             h[:, CN + t * TILE_N:CN + (t + 1) * TILE_N],
                             start=False, stop=False)
            nc.tensor.matmul(pv[:], W['fWv_app'][:], ef[:, cols],
                             start=False, stop=False)
            nc.tensor.matmul(pv[:], hvre[:, gtile, :], W['Etile'][:],
                             start=False, stop=True)
            hv = hp.tile([128, TILE_N], F32R, tag="fhv", bufs=2)
            if t % 2 == 0:
                nc.scalar.activation(hv[:], pv[:], AF.Relu)
            else:
                nc.vector.tensor_scalar(hv[:], pv[:], 0.0, None, op0=OP.max)

            prgb = psR.tile([3, TILE_N], F32, tag="rgbps")
            nc.tensor.matmul(prgb[:], W['Wrgb'][:], hv[:],
                             start=True, stop=True)
            nc.scalar.copy(rgbS[0:3, cols], prgb[:])
        nc.sync.dma_start(sb_[:], sigflat[:])
        nc.sync.dma_start(sigfT[r0:r0 + CHUNK_RAYS, :],
                          sb_[:].rearrange("a (p f) -> (a p) f", p=CHUNK_RAYS))
        rb_ = dram.tile([3, CN], F32, tag="rgbb")
        nc.sync.dma_start(rb_[:], rgbS[:])
        for cch in range(3):
            nc.sync.dma_start(
                rgb_rows[cch][r0:r0 + CHUNK_RAYS, :],
                rb_[cch:cch + 1, :].rearrange("a (p f) -> (a p) f",
                                              p=CHUNK_RAYS))
        if ci <= 3:
            pdf_batchB_fn(ci)

    # ======================= tail: composite =======================
    wf, _dzf = raw2w(sigfT[:, :], zf[:, 0:S], zf[:, 1:S + 1],
                     norm[:, :], bsig_f, R, "f")
    if debug:
        nc.sync.dma_start(dbg["d_sigf"][:], sigfT[:])
        nc.sync.dma_start(dbg["d_wf"][:], wf[:])
    rgbout = per.tile([R, 3], F32)
    for cch in range(3):
        sg = per.tile([R, S], F32, tag="sg%d" % cch)
        nc.scalar.activation(sg[:], rgb_rows[cch][:], AF.Sigmoid,
                             bias=float(brgb[cch]))
        nc.vector.tensor_tensor(sg[:], sg[:], wf[:], op=OP.mult)
        nc.vector.tensor_reduce(rgbout[:, cch:cch + 1], sg[:],
                                axis=mybir.AxisListType.X, op=OP.add)
    nc.sync.dma_start(OUT[:], rgbout[:])
    ctx.close()


# ---------------------------------------------------------------- entry
_CACHE = {}


def kernel(**inputs):
    inp = {k: np.asarray(v) for k, v in inputs.items()}
    consts, scal = host_prep(inp)
    key = (BUILD_STAGE, DEBUG_OUT, scal['pbo_f'], scal['bsig_f'],
           tuple(scal['brgb']))
    if key not in _CACHE:
        _CACHE[key] = build_nc(scal['pbo_f'], scal['bsig_f'], scal['brgb'],
                               stage=BUILD_STAGE, debug=DEBUG_OUT)
    nc = _CACHE[key]
    rays = np.asarray(inp['rays'], np.float32)
    in_maps = []
    for core in range(NCORES):
        m = {k: np.ascontiguousarray(v, dtype=np.float32)
             for k, v in consts.items()}
        m['rays'] = np.ascontiguousarray(rays[core * R:(core + 1) * R])
        in_maps.append(m)
    res = run_bass_kernel_spmd(nc, in_maps, core_ids=list(range(NCORES)))
    globals()['_LAST_RESULTS'] = res
    return np.concatenate([r['rgb_out'] for r in res.results], 0)
